# revision 4
# baseline (speedup 1.0000x reference)
"""AdaptIR forward as a Bass/Tile kernel on 8 TRN2 NeuronCores.

Data-parallel over batch N=8: one sample per core, all params replicated.
Self-contained: hardcodes all shapes; no sibling imports.

v2 layout: spectral pointwise packed 2 c-chunks onto 128 partitions,
depthwise conv on PE (diag stationaries), consolidated const DMAs,
half-angle atan2, bf16 output.

Partition placement convention (engines are lane-locked):
  rows 0:64  : xcp(padded head out), e_full/scr, WfRe, xcT1, W2, chunkA spectral
  rows 64:128: conv9, gx, ypb, WfIm, W3, tailT/siw consts, chunkB spectral
"""
import math
from contextlib import ExitStack

import ml_dtypes
import numpy as np

import concourse.bacc as bacc
import concourse.bass as bass
import concourse.mybir as mybir
import concourse.tile as tile
from concourse.bass_utils import run_bass_kernel_spmd

DT = mybir.dt.float32
BF = mybir.dt.bfloat16
AF = mybir.ActivationFunctionType
OP = mybir.AluOpType
AX = mybir.AxisListType

D, HID, F = 896, 64, 33
H = W = 64
HW = H * W              # 4096
NK = D // 128           # 7
NS = HID * F            # 2112
PI = math.pi
ERF_SCALE = 0.7071067811865476
EPS_IM = 1e-12

# spectral groups: (c0, nbA, nbB) -> chunk A rows 0:64, chunk B rows 64:128
GROUPS = [(0, 11, 11), (22, 11, 11), (44, 10, 10)]
GBASE = [0, 363, 726]
NSP = 1188


def _cols(widths):
    off, c = {}, 0
    for name, wd in widths:
        off[name] = c
        c += wd
    return off, c


F32_COLS, F32_W = _cols([
    ("b1", 1), ("pb1", 1), ("cb1", 1), ("cb2", 1), ("pb2", 1), ("cbvec", 1),
    ("pw1T", 32), ("pw2T", 64), ("cw1T", 16), ("cw2T", 64), ("pb2row", 64),
])
BF_COLS, BF_W = _cols([
    ("w1t", 448), ("fw2", 97), ("chm", 64), ("shm", 64), ("nshm", 64),
    ("fwi", 64), ("ident", 128), ("w9d", 576), ("tailT", 896),
    ("onesrow", 64), ("cwv", 1), ("siw", 1),
    ("awp", NSP), ("abp", NSP), ("pw2p", NSP), ("pb3p", NSP),
])


def build_graph(scalars, compile=True, trace_sim=False):
    nc = bacc.Bacc()
    x = nc.declare_dram_parameter("x", [HW, D], BF, isOutput=False)
    out = nc.declare_dram_parameter("out", [HW, D], BF, isOutput=True)
    cf32_d = nc.declare_dram_parameter("cf32", [128, F32_W], DT, isOutput=False)
    cbf_d = nc.declare_dram_parameter("cbf", [128, BF_W], BF, isOutput=False)

    for i, v in enumerate(sorted({scalars["compress_b"], scalars["si_b_eff"],
                                  EPS_IM, ERF_SCALE})):
        t = nc.alloc_sbuf_tensor(f"constap-{i}", [128, 1], DT)
        nc.gpsimd.memset(t.ap(), v)
        nc.const_aps.aps[(DT, v)] = t.ap()
    nc.all_engine_barrier()

    with tile.TileContext(nc, trace_sim=trace_sim) as tc, ExitStack() as ctx:
        cpool = ctx.enter_context(tc.tile_pool(name="consts", bufs=1))
        ps_t = ctx.enter_context(tc.tile_pool(name="ps_t", bufs=4, space="PSUM"))
        ps_tail = ctx.enter_context(tc.tile_pool(name="ps_tail", bufs=2, space="PSUM"))
        xTp = ctx.enter_context(tc.tile_pool(name="xT", bufs=10))
        persist = ctx.enter_context(tc.tile_pool(name="persist", bufs=1))
        spec = ctx.enter_context(tc.tile_pool(name="spec", bufs=16))
        outp = ctx.enter_context(tc.tile_pool(name="outp", bufs=2))
        sv = ctx.enter_context(tc.tile_pool(name="sv", bufs=1))

        cf32 = cpool.tile([128, F32_W], DT, tag="cf32")
        nc.sync.dma_start(out=cf32[:], in_=cf32_d[:])
        cbf = cpool.tile([128, BF_W], BF, tag="cbf")
        nc.sync.dma_start(out=cbf[:], in_=cbf_d[:])

        def f32c(name, rows, wd=1):
            c0 = F32_COLS[name]
            return cf32[rows[0]:rows[1], c0:c0 + wd]

        def bfc(name, rows, wd=1):
            c0 = BF_COLS[name]
            return cbf[rows[0]:rows[1], c0:c0 + wd]

        b1 = f32c("b1", (0, HID))
        pb1 = f32c("pb1", (0, 32))
        cb1 = f32c("cb1", (0, 16))
        cb2 = f32c("cb2", (64, 128))
        pb2 = f32c("pb2", (64, 128))
        cbvec = f32c("cbvec", (64, 128))
        pw1T = f32c("pw1T", (0, HID), 32)
        pw2T = f32c("pw2T", (0, 32), 64)
        cw1T = f32c("cw1T", (64, 128), 16)
        cw2T = f32c("cw2T", (0, 16), 64)
        pb2row = f32c("pb2row", (0, 1), 64)

        fw2 = bfc("fw2", (0, W), 97)
        chm_lo = bfc("chm", (0, 64), 64)
        shm_lo = bfc("shm", (0, 64), 64)
        shm_hi = bfc("shm", (64, 128), 64)
        chm_hi = bfc("chm", (64, 128), 64)
        nshm_lo = bfc("nshm", (0, 64), 64)
        nshm_hi = bfc("nshm", (64, 128), 64)
        fwi = bfc("fwi", (0, 66), 64)
        ident = bfc("ident", (0, 128), 128)
        tailT = bfc("tailT", (64, 128), D)
        onesrow = bfc("onesrow", (0, 1), 64)
        cwv = bfc("cwv", (0, HID))
        siw = bfc("siw", (64, 128))

        def w1tk(k):
            c0 = BF_COLS["w1t"] + 64 * k
            return cbf[:, c0:c0 + 64]

        def w9t(t_):
            c0 = BF_COLS["w9d"] + 64 * t_
            return cbf[0:HID, c0:c0 + 64]

        def specc(name, g, n, rows=(0, 128)):
            c0 = BF_COLS[name] + GBASE[g]
            return cbf[rows[0]:rows[1], c0:c0 + n]

        # ---- persistent SBUF tiles (64-row tensors packed in pairs) ----
        tA = persist.tile([128, 66 * 66], BF, tag="tA")   # xcp | conv9
        xcp = tA[0:HID, :]
        xcp_r = xcp.rearrange("p (h w) -> p h w", w=66)
        conv9 = tA[64:128, 0:HW]
        tB = persist.tile([128, HW], BF, tag="tB")        # scr | gx
        scr = tB[0:HID, :]
        gx = tB[64:128, :]
        tC = persist.tile([128, HW], BF, tag="tC")        # e_full | ypb
        e_full = tC[0:HID, :]
        ypb = tC[64:128, :]
        tD = persist.tile([HID, NS], BF, tag="tD")        # WfRe
        WfRe = tD[0:HID, :]
        tDi = persist.tile([HID, NS], BF, tag="tDi")      # WfIm (base 0!)
        WfIm = tDi[0:HID, :]
        tE = persist.tile([128, HW], BF, tag="tE")        # xcT1 | W3
        xcT1 = tE[0:HID, :]
        W3 = tE[64:128, :]
        WfP = persist.tile([128, HW], BF, tag="WfP")      # rows 0:97 used
        W2 = persist.tile([HID, HID * 66], BF, tag="W2")
        W2_r = W2.rearrange("p (c t) -> p c t", t=66)
        W2T = persist.tile([66, HW], BF, tag="W2T")
        e_row = persist.tile([1, HW], BF, tag="e_row")
        sgrow = persist.tile([1, HW], BF, tag="sgrow")

        drain_flip = [0]

        def drain(dst, src, bias=None):
            if drain_flip[0] % 2 == 0:
                if bias is None:
                    nc.scalar.activation(dst, src, AF.Copy)
                else:
                    nc.scalar.activation(dst, src, AF.Identity, bias=bias)
            else:
                if bias is None:
                    nc.vector.tensor_copy(dst, src)
                else:
                    nc.vector.tensor_scalar(dst, src, bias, None, OP.add)
            drain_flip[0] += 1

        # ---- zero the 1-px border of the padded conv buffer ----
        nc.gpsimd.memset(xcp_r[:, 0, :], 0.0)
        nc.gpsimd.memset(xcp_r[:, 65, :], 0.0)
        nc.gpsimd.memset(xcp_r[:, 1:65, 0:1], 0.0)
        nc.gpsimd.memset(xcp_r[:, 1:65, 65:66], 0.0)

        # ---- x loads: 14 transpose DMAs of [2048, 128] ----
        xt = {}
        for jh in range(2):
            for k in range(NK):
                t = xTp.tile([128, 2048], BF, tag="xt", name=f"xt_{jh}_{k}")
                nc.sync.dma_start(
                    out=t[:],
                    in_=x[jh * 2048:(jh + 1) * 2048, k * 128:(k + 1) * 128],
                    transpose=True)
                xt[(jh, k)] = t

        # ---- A: head matmul, drain straight into padded xcp ----
        for j in range(8):
            jh, jl = j // 4, j % 4
            ps_h = ps_t.tile([HID, 512], DT, tag="pst")
            for k in range(NK):
                nc.tensor.matmul(ps_h[:], w1tk(k),
                                 xt[(jh, k)][:, jl * 512:(jl + 1) * 512],
                                 start=(k == 0), stop=(k == NK - 1))
            drain(xcp_r[:, 1 + 8 * j:9 + 8 * j, 1:65], ps_h[:], bias=b1)

        # ---- B part 1: compress -> exp (+Z accum) -> e broadcast ----
        z8 = sv.tile([1, 8], DT, tag="z8")
        for j in range(8):
            ps1 = ps_t.tile([1, 512], DT, tag="pst")
            nc.tensor.matmul(ps1[:], cwv,
                             xcp_r[:, 1 + 8 * j:9 + 8 * j, 1:65],
                             start=True, stop=True)
            nc.scalar.activation(e_row[:, j * 512:(j + 1) * 512], ps1[:],
                                 AF.Exp, bias=scalars["compress_b"],
                                 accum_out=z8[:, j:j + 1])
            psb = ps_t.tile([HID, 512], DT, tag="pst")
            nc.tensor.matmul(psb[:], onesrow,
                             e_row[:, j * 512:(j + 1) * 512], start=True, stop=True)
            drain(e_full[:, j * 512:(j + 1) * 512], psb[:])

        # ---- T1: per-h transpose of xcp -> xcT1 [w, (h,c)] ----
        for hb in range(8):
            pst = ps_t.tile([128, 512], BF, tag="pst")
            for r in range(8):
                hh = hb * 8 + r
                nc.tensor.transpose(pst[:W, r * 64:(r + 1) * 64],
                                    xcp_r[:, 1 + hh, 1:65],
                                    ident[0:HID, 0:HID])
            drain(xcT1[:, hb * 512:(hb + 1) * 512], pst[:W, :])

        # ---- FFT-W ----
        for j in range(8):
            psf = ps_t.tile([97, 512], DT, tag="pst")
            nc.tensor.matmul(psf[:], fw2, xcT1[:, j * 512:(j + 1) * 512],
                             start=True, stop=True)
            drain(WfP[0:97, j * 512:(j + 1) * 512], psf[:])

        # ---- T2: [97,(h,c)] -> WfRe[h,(c,f)] rows 0:64, WfIm rows 64:128 ----
        WfP_r = WfP[0:97, :].rearrange("p (h c) -> p c h", c=HID)
        for g in range(8):
            pstRI = ps_t.tile([128, 272], BF, tag="pst")
            for r in range(8):
                c = g * 8 + r
                nc.tensor.transpose(pstRI[0:64, r * 34:r * 34 + F],
                                    WfP_r[0:F, c, :], ident[0:F, 0:F])
                nc.tensor.transpose(pstRI[64:128, r * 34:r * 34 + F],
                                    WfP_r[64:97, c, :], ident[64:97, 64:97])
            pv = pstRI.rearrange("p (c t) -> p c t", t=34)[:, :, 0:F]
            ov = tD[:, g * 8 * F:(g + 1) * 8 * F].rearrange("p (c t) -> p c t", t=F)
            drain(ov, pv)

        # ---- B part 2: pooled + proj MLP front ----
        scr_r = scr.rearrange("p (h w) -> p h w", w=W)
        ef_r = e_full.rearrange("p (h w) -> p h w", w=W)
        nc.vector.tensor_tensor(scr_r[:], xcp_r[:, 1:65, 1:65], ef_r[:], OP.mult)
        praw = sv.tile([HID, 1], DT, tag="praw")
        nc.vector.reduce_sum(praw[:], scr[:], AX.X)
        z8b = sv.tile([1, 8], BF, tag="z8b")
        nc.scalar.activation(z8b[:], z8[:], AF.Copy)
        zps = ps_t.tile([HID, 8], DT, tag="pst")
        nc.tensor.matmul(zps[:], onesrow, z8b[:], start=True, stop=True)
        zb = sv.tile([HID, 8], DT, tag="zb")
        nc.vector.tensor_copy(zb[:], zps[:])
        Zv = sv.tile([HID, 1], DT, tag="Zv")
        nc.vector.reduce_sum(Zv[:], zb[:], AX.X)
        zr = sv.tile([HID, 1], DT, tag="zr")
        nc.vector.reciprocal(zr[:], Zv[:])
        pooled = sv.tile([HID, 1], DT, tag="pooled")
        nc.vector.tensor_tensor(pooled[:], praw[:], zr[:], OP.mult)
        psm = ps_t.tile([32, 1], DT, tag="pst")
        nc.tensor.matmul(psm[:], pw1T, pooled[:], start=True, stop=True)
        hv = sv.tile([32, 1], DT, tag="hv")
        nc.scalar.activation(hv[:], psm[:], AF.Identity, bias=pb1)

        # ---- FFT-H forward (packed) + mag cluster for all groups ----
        gd = []
        for g, (c0, nA, nB) in enumerate(GROUPS):
            n = nA * F
            packed = nB > 0
            rows = 128 if packed else 64
            slA = slice(c0 * F, c0 * F + n)
            slB = slice((c0 + nA) * F, (c0 + nA) * F + nB * F)

            psRe = ps_t.tile([128, 512], DT, tag="pst", name=f"psRe{g}")
            psIm = ps_t.tile([128, 512], DT, tag="pst", name=f"psIm{g}")
            nc.tensor.matmul(psRe[0:64, 0:n], chm_lo, WfRe[:, slA],
                             start=True, stop=False)
            nc.tensor.matmul(psRe[0:64, 0:n], shm_hi, WfIm[:, slA],
                             start=False, stop=True)
            nc.tensor.matmul(psIm[0:64, 0:n], chm_hi, WfIm[:, slA],
                             start=True, stop=False)
            nc.tensor.matmul(psIm[0:64, 0:n], nshm_lo, WfRe[:, slA],
                             start=False, stop=True)
            if packed:
                nc.tensor.matmul(psRe[64:128, 0:n], chm_lo, WfRe[:, slB],
                                 start=True, stop=False)
                nc.tensor.matmul(psRe[64:128, 0:n], shm_hi, WfIm[:, slB],
                                 start=False, stop=True)
                nc.tensor.matmul(psIm[64:128, 0:n], chm_hi, WfIm[:, slB],
                                 start=True, stop=False)
                nc.tensor.matmul(psIm[64:128, 0:n], nshm_lo, WfRe[:, slB],
                                 start=False, stop=True)
            ReG = spec.tile([rows, n], DT, tag="sp", name=f"ReG{g}")
            nc.scalar.activation(ReG[:], psRe[0:rows, 0:n], AF.Copy)
            ImG = spec.tile([rows, n], DT, tag="sp", name=f"ImG{g}")
            nc.scalar.activation(ImG[:], psIm[0:rows, 0:n], AF.Identity,
                                 bias=EPS_IM)
            sqR = spec.tile([rows, n], DT, tag="sp", name=f"sqR{g}")
            nc.scalar.activation(sqR[:], ReG[:], AF.Square)
            sqI = spec.tile([rows, n], DT, tag="sp", name=f"sqI{g}")
            nc.scalar.activation(sqI[:], ImG[:], AF.Square)
            m2 = spec.tile([rows, n], DT, tag="sp", name=f"m2{g}")
            nc.vector.tensor_tensor(m2[:], sqR[:], sqI[:], OP.add)
            mag = spec.tile([rows, n], DT, tag="sp", name=f"mag{g}")
            nc.scalar.activation(mag[:], m2[:], AF.Sqrt)
            gd.append((n, rows, ReG, ImG, mag))

        # ---- spectral chains (trig table) + inverse + conv interleave ----
        def conv_chunk(j):
            pc = ps_t.tile([128, 512], DT, tag="pst", name=f"conv{j}")
            for t_ in range(9):
                dy, dx = t_ // 3, t_ % 3
                nc.tensor.matmul(pc[64:128, :], w9t(t_),
                                 xcp_r[:, 8 * j + dy:8 * j + 8 + dy, dx:dx + 64],
                                 start=(t_ == 0), stop=(t_ == 8))
            drain(conv9[:, j * 512:(j + 1) * 512], pc[64:128, :])

        for g, (c0, nA, nB) in enumerate(GROUPS):
            n, rows, ReG, ImG, mag = gd[g]

            def ct(name, dtype=DT):
                return spec.tile([rows, n], dtype, tag="sp",
                                 name=f"{name}{g}")[:]

            den = ct("den")
            nc.vector.tensor_tensor(den, mag[:], ReG[:], OP.add)
            den2 = ct("den2")
            nc.vector.tensor_scalar(den2, den, 1e-30, None, OP.max)
            dri = ct("dri")
            nc.vector.reciprocal(dri, den2)
            q = ct("q")
            nc.vector.tensor_tensor(q, ImG[:], dri, OP.mult)
            aq = ct("aq")
            nc.scalar.activation(aq, q, AF.Arctan)
            vp1 = ct("vp1")
            nc.vector.tensor_tensor(vp1, aq, specc("pw2p", g, n, (0, rows)),
                                    OP.mult)
            vpre = ct("vpre")
            nc.vector.tensor_tensor(vpre, vp1, specc("pb3p", g, n, (0, rows)),
                                    OP.add)
            sinv = ct("sinv")
            nc.scalar.activation(sinv, vpre, AF.Sin)
            k3 = ct("k3")
            nc.vector.tensor_scalar(k3, vpre, PI / 2, 2 * PI, OP.is_gt, OP.mult)
            cos_in = ct("cos_in")
            nc.vector.scalar_tensor_tensor(cos_in, vpre, PI / 2, k3,
                                           OP.add, OP.subtract)
            cosv = ct("cosv")
            nc.scalar.activation(cosv, cos_in, AF.Sin)
            magw = ct("magw")
            nc.vector.tensor_tensor(magw, mag[:], specc("awp", g, n, (0, rows)),
                                    OP.mult)
            mag2 = ct("mag2")
            nc.vector.tensor_tensor(mag2, magw, specc("abp", g, n, (0, rows)),
                                    OP.add)
            Rp = ct("Rp", BF)
            nc.vector.tensor_tensor(Rp, mag2, cosv, OP.mult)
            Ip = ct("Ip", BF)
            nc.vector.tensor_tensor(Ip, mag2, sinv, OP.mult)

            # inverse FFT-H for chunk A (rows 0:64) and chunk B (rows 64:128)
            halves = [(slice(0, 64), c0, nA)]
            if nB > 0:
                halves.append((slice(64, 128), c0 + nA, nB))
            for hs, cc0, nb in halves:
                nn = nb * F
                psR = ps_t.tile([64, 512], DT, tag="pst", name=f"ivR{g}{cc0}")
                lhs_c = chm_lo if hs.start == 0 else chm_hi
                lhs_ns = nshm_lo if hs.start == 0 else nshm_hi
                lhs_s = bfc("shm", (0, 64), 64) if hs.start == 0 else shm_hi
                nc.tensor.matmul(psR[:, 0:nn], lhs_c, Rp[hs, 0:nn],
                                 start=True, stop=False)
                nc.tensor.matmul(psR[:, 0:nn], lhs_ns, Ip[hs, 0:nn],
                                 start=False, stop=True)
                nc.scalar.activation(W2_r[:, cc0:cc0 + nb, 0:F],
                                     psR[:, 0:nn].rearrange("p (c t) -> p c t", t=F),
                                     AF.Copy)
                psI = ps_t.tile([64, 512], DT, tag="pst", name=f"ivI{g}{cc0}")
                nc.tensor.matmul(psI[:, 0:nn], lhs_c, Ip[hs, 0:nn],
                                 start=True, stop=False)
                nc.tensor.matmul(psI[:, 0:nn], lhs_s, Rp[hs, 0:nn],
                                 start=False, stop=True)
                nc.vector.tensor_copy(W2_r[:, cc0:cc0 + nb, F:66],
                                      psI[:, 0:nn].rearrange("p (c t) -> p c t", t=F))
            # interleave conv chunks so PE fills DVE-chain shadows
            for j in range(g * 3, min(g * 3 + 3, 8)):
                conv_chunk(j)

        # ---- W2T + irfft-W -> W3 (rows 64:128) ----
        for g in range(8):
            pst = ps_t.tile([128, 512], BF, tag="pst")
            for r in range(8):
                c = g * 8 + r
                nc.tensor.transpose(pst[0:66, r * 64:(r + 1) * 64],
                                    W2[:, c * 66:(c + 1) * 66], ident[0:H, 0:H])
            drain(W2T[:, g * 512:(g + 1) * 512], pst[0:66, :])
        for j in range(8):
            psw = ps_t.tile([128, 512], DT, tag="pst")
            nc.tensor.matmul(psw[64:128, :], fwi, W2T[:, j * 512:(j + 1) * 512],
                             start=True, stop=True)
            drain(W3[:, j * 512:(j + 1) * 512], psw[64:128, :])

        # ---- T4: W3 [w,(c,h)] -> gx [c,(h,w)] rows 64:128, with |.| ----
        W3_r = W3.rearrange("p (c h) -> p h c", h=H)
        avg8 = sv.tile([128, 8], DT, tag="avg8")
        ident_hi = ident[64:128, 64:128]
        for hb in range(8):
            pst = ps_t.tile([128, 512], BF, tag="pst")
            for r in range(8):
                hh = hb * 8 + r
                nc.tensor.transpose(pst[64:128, r * 64:(r + 1) * 64],
                                    W3_r[:, hh, :], ident_hi)
            nc.scalar.activation(gx[:, hb * 512:(hb + 1) * 512], pst[64:128, :],
                                 AF.Abs, accum_out=avg8[64:128, hb:hb + 1])
        avgn = sv.tile([128, 1], DT, tag="avgn")
        nc.vector.reduce_sum(avgn[64:128, :], avg8[64:128, :], AX.X)
        avgv = sv.tile([128, 1], DT, tag="avgv")
        nc.vector.tensor_scalar(avgv[64:128, :], avgn[64:128, :], 1.0 / HW,
                                None, OP.mult)

        # ---- proj MLP finish (erf gelu) -> cs row + cs col(hi) ----
        e1 = sv.tile([32, 1], DT, tag="e1")
        nc.scalar.activation(e1[:], hv[:], AF.Erf, scale=ERF_SCALE)
        gh = sv.tile([32, 1], DT, tag="gh")
        nc.vector.tensor_scalar(gh[:], e1[:], 1.0, 0.5, OP.add, OP.mult)
        g1v = sv.tile([32, 1], DT, tag="g1v")
        nc.vector.tensor_tensor(g1v[:], hv[:], gh[:], OP.mult)
        psm2 = ps_t.tile([128, 1], DT, tag="pst")
        nc.tensor.matmul(psm2[64:128, :], pw2T, g1v[:], start=True, stop=True)
        csb = sv.tile([128, 1], DT, tag="csb")
        nc.scalar.activation(csb[64:128, :], psm2[64:128, :], AF.Identity,
                             bias=pb2)
        psr = ps_t.tile([1, 64], DT, tag="pst")
        nc.tensor.matmul(psr[:], g1v[:], pw2T, start=True, stop=True)
        csr = sv.tile([1, 64], BF, tag="csr")
        nc.vector.tensor_tensor(csr[:], psr[:], pb2row, OP.add)

        # ---- channel gate MLP (rows 64:128) ----
        psc = ps_t.tile([16, 1], DT, tag="pst")
        nc.tensor.matmul(psc[:], cw1T, avgv[64:128, :], start=True, stop=True)
        chv = sv.tile([16, 1], DT, tag="chv")
        nc.scalar.activation(chv[:], psc[:], AF.Identity, bias=cb1)
        ce1 = sv.tile([16, 1], DT, tag="ce1")
        nc.scalar.activation(ce1[:], chv[:], AF.Erf, scale=ERF_SCALE)
        cgh = sv.tile([16, 1], DT, tag="cgh")
        nc.vector.tensor_scalar(cgh[:], ce1[:], 1.0, 0.5, OP.add, OP.mult)
        cg1 = sv.tile([16, 1], DT, tag="cg1")
        nc.vector.tensor_tensor(cg1[:], chv[:], cgh[:], OP.mult)
        psc2 = ps_t.tile([128, 1], DT, tag="pst")
        nc.tensor.matmul(psc2[64:128, :], cw2T, cg1[:], start=True, stop=True)
        cgb = sv.tile([128, 1], DT, tag="cgb")
        nc.scalar.activation(cgb[64:128, :], psc2[64:128, :], AF.Sigmoid,
                             bias=cb2)
        cscg = sv.tile([128, 1], DT, tag="cscg")
        nc.vector.tensor_tensor(cscg[64:128, :], csb[64:128, :], cgb[64:128, :],
                                OP.mult)
        bstar = sv.tile([128, 1], DT, tag="bstar")
        nc.vector.tensor_tensor(bstar[64:128, :], cscg[64:128, :], cbvec,
                                OP.mult)

        # ---- spatial gate + y assembly (all rows 64:128) ----
        for j in range(8):
            ps1 = ps_t.tile([1, 512], DT, tag="pst")
            nc.tensor.matmul(ps1[:], siw, conv9[:, j * 512:(j + 1) * 512],
                             start=True, stop=True)
            nc.scalar.activation(sgrow[:, j * 512:(j + 1) * 512], ps1[:],
                                 AF.Sigmoid, bias=scalars["si_b_eff"])
            psb = ps_t.tile([128, 512], DT, tag="pst")
            nc.tensor.matmul(psb[64:128, :], csr[:],
                             sgrow[:, j * 512:(j + 1) * 512], start=True, stop=True)
            y1 = spec.tile([128, 512], BF, tag="sp", name=f"y1_{j}")
            nc.vector.tensor_tensor(y1[64:128, :], gx[:, j * 512:(j + 1) * 512],
                                    psb[64:128, :], OP.mult)
            tl = spec.tile([128, 512], BF, tag="sp", name=f"tl_{j}")
            nc.scalar.activation(tl[64:128, :], conv9[:, j * 512:(j + 1) * 512],
                                 AF.Identity, scale=cscg[64:128, :],
                                 bias=bstar[64:128, :])
            nc.vector.tensor_tensor(ypb[:, j * 512:(j + 1) * 512],
                                    y1[64:128, :], tl[64:128, :], OP.add)

        # ---- tail: out[hw, D] = ypb^T @ tailT, bf16 out in 4-chunk DMAs ----
        out_r = out.rearrange("(g i p) d -> g p i d", i=4, p=128)
        for gq in range(8):
            osb = outp.tile([128, 4 * D], BF, tag="osb")
            for i4 in range(4):
                i = gq * 4 + i4
                pst_ = ps_tail.tile([128, D], DT, tag="tacc")
                nc.tensor.matmul(pst_[:, 0:512], ypb[:, i * 128:(i + 1) * 128],
                                 tailT[:, 0:512], start=True, stop=True)
                nc.tensor.matmul(pst_[:, 512:D], ypb[:, i * 128:(i + 1) * 128],
                                 tailT[:, 512:D], start=True, stop=True)
                drain(osb[:, i4 * D:(i4 + 1) * D], pst_[:])
            nc.sync.dma_start(out=out_r[gq], in_=osb.rearrange("p (i d) -> p i d", d=D))

    if compile:
        nc.compile()
    return nc


def host_prep(inp):
    p = {k: np.ascontiguousarray(np.asarray(v, np.float32)) for k, v in inp.items()}
    s = p["bn_w"] / np.sqrt(p["bn_var"] + 1e-5)
    W1 = (p["head_w"] * s[:, None]).astype(np.float64)
    b1 = (p["head_b"] - p["bn_mean"]) * s + p["bn_b"]
    w = np.arange(W)
    f = np.arange(F)
    h = np.arange(H)
    Cw = np.cos(2 * np.pi * np.outer(w, f) / W) / 8.0
    Sw = -np.sin(2 * np.pi * np.outer(w, f) / W) / 8.0
    Fw2 = np.concatenate([Cw, np.zeros((W, 31)), Sw], axis=1)      # [64, 97]
    Ch = np.cos(2 * np.pi * np.outer(h, h) / H) / 8.0
    Sh = np.sin(2 * np.pi * np.outer(h, h) / H) / 8.0
    cf = np.full(F, 2.0)
    cf[0] = 1.0
    cf[F - 1] = 1.0
    Fwi = np.concatenate([
        (cf[:, None] * np.cos(2 * np.pi * np.outer(f, w) / W)) / 8.0,
        -(cf[:, None] * np.sin(2 * np.pi * np.outer(f, w) / W)) / 8.0,
    ], axis=0)                                                     # [66, 64]
    w9 = (p["conv_A"] @ p["conv_B"])                               # [64, 9]

    cf32 = np.zeros((128, F32_W), np.float32)

    def put32(name, rows, arr):
        arr = np.asarray(arr, np.float32)
        if arr.ndim == 1:
            arr = arr[:, None]
        cf32[rows[0]:rows[0] + arr.shape[0],
             F32_COLS[name]:F32_COLS[name] + arr.shape[1]] = arr

    put32("b1", (0,), b1)
    put32("pb1", (0,), p["proj_b1"])
    put32("cb1", (0,), p["ci_b1"])
    put32("cb2", (64,), p["ci_b2"])
    put32("pb2", (64,), p["proj_b2"])
    put32("cbvec", (64,), p["conv_bias"])
    put32("pw1T", (0,), np.ascontiguousarray(p["proj_w1"].T))
    put32("pw2T", (0,), np.ascontiguousarray(p["proj_w2"].T))
    put32("cw1T", (64,), np.ascontiguousarray(p["ci_w1"].T))
    put32("cw2T", (0,), np.ascontiguousarray(p["ci_w2"].T))
    put32("pb2row", (0,), p["proj_b2"][None, :])

    cbf = np.zeros((128, BF_W), np.float64)

    def putbf(name, rows, arr):
        arr = np.asarray(arr, np.float64)
        if arr.ndim == 1:
            arr = arr[:, None]
        cbf[rows[0]:rows[0] + arr.shape[0],
            BF_COLS[name]:BF_COLS[name] + arr.shape[1]] = arr

    putbf("w1t", (0,), np.ascontiguousarray(W1.T).reshape(NK, 128, HID)
          .transpose(1, 0, 2).reshape(128, NK * HID))
    putbf("fw2", (0,), Fw2)
    putbf("chm", (0,), Ch)
    putbf("chm", (64,), Ch)
    putbf("shm", (0,), Sh)
    putbf("shm", (64,), Sh)
    putbf("nshm", (0,), -Sh)
    putbf("nshm", (64,), -Sh)
    putbf("fwi", (0,), Fwi)
    putbf("ident", (0,), np.eye(128))
    w9d = np.zeros((64, 9 * 64))
    for t_ in range(9):
        w9d[:, t_ * 64:(t_ + 1) * 64] = np.diag(w9[:, t_])
    putbf("w9d", (0,), w9d)
    putbf("tailT", (64,), np.ascontiguousarray(p["tail_w"].T))
    putbf("onesrow", (0,), np.ones((1, 64)))
    putbf("cwv", (0,), np.ascontiguousarray(p["compress_w"].T))
    putbf("siw", (64,), np.ascontiguousarray(p["si_w"].T))

    def spack(name, vec):
        buf = np.zeros((128, NSP))
        for g, (c0, nA, nB) in enumerate(GROUPS):
            base = GBASE[g]
            buf[0:64, base:base + nA * F] = np.repeat(vec[c0:c0 + nA], F)[None, :]
            if nB > 0:
                buf[64:128, base:base + nB * F] = \
                    np.repeat(vec[c0 + nA:c0 + nA + nB], F)[None, :]
        putbf(name, (0,), buf)

    spack("awp", p["amp_w"])
    spack("abp", p["amp_b"])
    spack("pw2p", 2.0 * p["pha_w"])
    spack("pb3p", p["pha_b"])

    consts = dict(
        cf32=np.ascontiguousarray(cf32),
        cbf=np.ascontiguousarray(cbf.astype(ml_dtypes.bfloat16)),
    )
    scalars = dict(
        compress_b=float(p["compress_b"][0]),
        si_b_eff=float(p["si_b"][0] + p["si_w"][0] @ p["conv_bias"]),
    )
    return consts, scalars


def kernel(**inputs):
    x = np.asarray(inputs["x"], np.float32)
    N = x.shape[0]
    consts, scalars = host_prep(inputs)
    nc = build_graph(scalars)
    in_maps = []
    for i in range(N):
        xu = x[i].reshape(HW, D).view(np.uint32)
        xr = ((xu + 0x7FFF + ((xu >> 16) & 1)) >> 16).astype(np.uint16)  # bf16 RNE
        m = {"x": xr.view(ml_dtypes.bfloat16)}
        m.update(consts)
        in_maps.append(m)
    res = run_bass_kernel_spmd(nc, in_maps, core_ids=list(range(N)))
    outs = np.stack([np.asarray(res.results[i]["out"], np.float32).reshape(H, W, D)
                     for i in range(N)])
    return outs.astype(np.float32)


# revision 5
# speedup vs baseline: 1.0045x; 1.0045x over previous
"""AdaptIR forward as a Bass/Tile kernel on 8 TRN2 NeuronCores.

Data-parallel over batch N=8: one sample per core, all params replicated.
Self-contained: hardcodes all shapes; no sibling imports.

v2 layout: spectral pointwise packed 2 c-chunks onto 128 partitions,
depthwise conv on PE (diag stationaries), consolidated const DMAs,
half-angle atan2, bf16 output.

Partition placement convention (engines are lane-locked):
  rows 0:64  : xcp(padded head out), e_full/scr, WfRe, xcT1, W2, chunkA spectral
  rows 64:128: conv9, gx, ypb, WfIm, W3, tailT/siw consts, chunkB spectral
"""
import math
from contextlib import ExitStack

import ml_dtypes
import numpy as np

import concourse.bacc as bacc
import concourse.bass as bass
import concourse.mybir as mybir
import concourse.tile as tile
from concourse.bass_utils import run_bass_kernel_spmd

DT = mybir.dt.float32
BF = mybir.dt.bfloat16
AF = mybir.ActivationFunctionType
OP = mybir.AluOpType
AX = mybir.AxisListType

D, HID, F = 896, 64, 33
H = W = 64
HW = H * W              # 4096
NK = D // 128           # 7
NS = HID * F            # 2112
PI = math.pi
ERF_SCALE = 0.7071067811865476
EPS_IM = 1e-12

# spectral groups: (c0, nbA, nbB) -> chunk A rows 0:64, chunk B rows 64:128
GROUPS = [(0, 12, 12), (24, 12, 12), (48, 8, 8)]
GBASE = [0, 396, 792]
NSP = 1188


def _cols(widths):
    off, c = {}, 0
    for name, wd in widths:
        off[name] = c
        c += wd
    return off, c


F32_COLS, F32_W = _cols([
    ("b1", 1), ("pb1", 1), ("cb1", 1), ("cb2", 1), ("pb2", 1), ("cbvec", 1),
    ("pw1T", 32), ("pw2T", 64), ("cw1T", 16), ("cw2T", 64), ("pb2row", 64),
])
BF_COLS, BF_W = _cols([
    ("w1t", 448), ("fw2", 97), ("chm", 64), ("shm", 64), ("nshm", 64),
    ("fwi", 64), ("ident", 128), ("w9d", 576), ("tailT", 896),
    ("onesrow", 64), ("cwv", 1), ("siw", 1),
    ("awp", NSP), ("abp", NSP), ("pw2p", NSP), ("pb3p", NSP),
])


def build_graph(scalars, compile=True, trace_sim=False):
    nc = bacc.Bacc()
    x = nc.declare_dram_parameter("x", [HW, D], BF, isOutput=False)
    out = nc.declare_dram_parameter("out", [HW, D], BF, isOutput=True)
    cf32_d = nc.declare_dram_parameter("cf32", [128, F32_W], DT, isOutput=False)
    cbf_d = nc.declare_dram_parameter("cbf", [128, BF_W], BF, isOutput=False)

    for i, v in enumerate(sorted({scalars["compress_b"], scalars["si_b_eff"],
                                  EPS_IM, ERF_SCALE})):
        t = nc.alloc_sbuf_tensor(f"constap-{i}", [128, 1], DT)
        nc.gpsimd.memset(t.ap(), v)
        nc.const_aps.aps[(DT, v)] = t.ap()
    nc.all_engine_barrier()

    with tile.TileContext(nc, trace_sim=trace_sim) as tc, ExitStack() as ctx:
        cpool = ctx.enter_context(tc.tile_pool(name="consts", bufs=1))
        ps_t = ctx.enter_context(tc.tile_pool(name="ps_t", bufs=4, space="PSUM"))
        ps_tail = ctx.enter_context(tc.tile_pool(name="ps_tail", bufs=2, space="PSUM"))
        xTp = ctx.enter_context(tc.tile_pool(name="xT", bufs=10))
        persist = ctx.enter_context(tc.tile_pool(name="persist", bufs=1))
        spec = ctx.enter_context(tc.tile_pool(name="spec", bufs=16))
        outp = ctx.enter_context(tc.tile_pool(name="outp", bufs=2))
        sv = ctx.enter_context(tc.tile_pool(name="sv", bufs=1))

        cf32 = cpool.tile([128, F32_W], DT, tag="cf32")
        nc.sync.dma_start(out=cf32[:], in_=cf32_d[:])
        cbf = cpool.tile([128, BF_W], BF, tag="cbf")
        nc.sync.dma_start(out=cbf[:], in_=cbf_d[:])

        def f32c(name, rows, wd=1):
            c0 = F32_COLS[name]
            return cf32[rows[0]:rows[1], c0:c0 + wd]

        def bfc(name, rows, wd=1):
            c0 = BF_COLS[name]
            return cbf[rows[0]:rows[1], c0:c0 + wd]

        b1 = f32c("b1", (0, HID))
        pb1 = f32c("pb1", (0, 32))
        cb1 = f32c("cb1", (0, 16))
        cb2 = f32c("cb2", (64, 128))
        pb2 = f32c("pb2", (64, 128))
        cbvec = f32c("cbvec", (64, 128))
        pw1T = f32c("pw1T", (0, HID), 32)
        pw2T = f32c("pw2T", (0, 32), 64)
        cw1T = f32c("cw1T", (64, 128), 16)
        cw2T = f32c("cw2T", (0, 16), 64)
        pb2row = f32c("pb2row", (0, 1), 64)

        fw2 = bfc("fw2", (0, W), 97)
        chm_lo = bfc("chm", (0, 64), 64)
        shm_lo = bfc("shm", (0, 64), 64)
        shm_hi = bfc("shm", (64, 128), 64)
        chm_hi = bfc("chm", (64, 128), 64)
        nshm_lo = bfc("nshm", (0, 64), 64)
        nshm_hi = bfc("nshm", (64, 128), 64)
        fwi = bfc("fwi", (0, 66), 64)
        ident = bfc("ident", (0, 128), 128)
        tailT = bfc("tailT", (64, 128), D)
        onesrow = bfc("onesrow", (0, 1), 64)
        cwv = bfc("cwv", (0, HID))
        siw = bfc("siw", (64, 128))

        def w1tk(k):
            c0 = BF_COLS["w1t"] + 64 * k
            return cbf[:, c0:c0 + 64]

        def w9t(t_):
            c0 = BF_COLS["w9d"] + 64 * t_
            return cbf[0:HID, c0:c0 + 64]

        def specc(name, g, n, rows=(0, 128)):
            c0 = BF_COLS[name] + GBASE[g]
            return cbf[rows[0]:rows[1], c0:c0 + n]

        # ---- persistent SBUF tiles (64-row tensors packed in pairs) ----
        tA = persist.tile([128, 66 * 66], BF, tag="tA")   # xcp | conv9
        xcp = tA[0:HID, :]
        xcp_r = xcp.rearrange("p (h w) -> p h w", w=66)
        conv9 = tA[64:128, 0:HW]
        tB = persist.tile([128, HW], BF, tag="tB")        # scr | gx
        scr = tB[0:HID, :]
        gx = tB[64:128, :]
        tC = persist.tile([128, HW], BF, tag="tC")        # e_full | ypb
        e_full = tC[0:HID, :]
        ypb = tC[64:128, :]
        tD = persist.tile([HID, NS], BF, tag="tD")        # WfRe
        WfRe = tD[0:HID, :]
        tDi = persist.tile([HID, NS], BF, tag="tDi")      # WfIm (base 0!)
        WfIm = tDi[0:HID, :]
        tE = persist.tile([128, HW], BF, tag="tE")        # xcT1 | W3
        xcT1 = tE[0:HID, :]
        W3 = tE[64:128, :]
        WfP = persist.tile([128, HW], BF, tag="WfP")      # rows 0:97 used
        W2 = persist.tile([HID, HID * 66], BF, tag="W2")
        W2_r = W2.rearrange("p (c t) -> p c t", t=66)
        W2T = persist.tile([66, HW], BF, tag="W2T")
        e_row = persist.tile([1, HW], BF, tag="e_row")
        sgrow = persist.tile([1, HW], BF, tag="sgrow")

        drain_flip = [0]

        def drain(dst, src, bias=None):
            if drain_flip[0] % 2 == 0:
                if bias is None:
                    nc.scalar.activation(dst, src, AF.Copy)
                else:
                    nc.scalar.activation(dst, src, AF.Identity, bias=bias)
            else:
                if bias is None:
                    nc.vector.tensor_copy(dst, src)
                else:
                    nc.vector.tensor_scalar(dst, src, bias, None, OP.add)
            drain_flip[0] += 1

        # ---- zero the 1-px border of the padded conv buffer ----
        nc.gpsimd.memset(xcp_r[:, 0, :], 0.0)
        nc.gpsimd.memset(xcp_r[:, 65, :], 0.0)
        nc.gpsimd.memset(xcp_r[:, 1:65, 0:1], 0.0)
        nc.gpsimd.memset(xcp_r[:, 1:65, 65:66], 0.0)

        # ---- x loads: 14 transpose DMAs of [2048, 128] ----
        xt = {}
        for jh in range(2):
            for k in range(NK):
                t = xTp.tile([128, 2048], BF, tag="xt", name=f"xt_{jh}_{k}")
                nc.sync.dma_start(
                    out=t[:],
                    in_=x[jh * 2048:(jh + 1) * 2048, k * 128:(k + 1) * 128],
                    transpose=True)
                xt[(jh, k)] = t

        # ---- A: head matmul, drain straight into padded xcp ----
        for j in range(8):
            jh, jl = j // 4, j % 4
            ps_h = ps_t.tile([HID, 512], DT, tag="pst")
            for k in range(NK):
                nc.tensor.matmul(ps_h[:], w1tk(k),
                                 xt[(jh, k)][:, jl * 512:(jl + 1) * 512],
                                 start=(k == 0), stop=(k == NK - 1))
            drain(xcp_r[:, 1 + 8 * j:9 + 8 * j, 1:65], ps_h[:], bias=b1)

        # ---- B part 1: compress -> exp (+Z accum) -> e broadcast ----
        z8 = sv.tile([1, 8], DT, tag="z8")
        for j in range(8):
            ps1 = ps_t.tile([1, 512], DT, tag="pst")
            nc.tensor.matmul(ps1[:], cwv,
                             xcp_r[:, 1 + 8 * j:9 + 8 * j, 1:65],
                             start=True, stop=True)
            nc.scalar.activation(e_row[:, j * 512:(j + 1) * 512], ps1[:],
                                 AF.Exp, bias=scalars["compress_b"],
                                 accum_out=z8[:, j:j + 1])
            psb = ps_t.tile([HID, 512], DT, tag="pst")
            nc.tensor.matmul(psb[:], onesrow,
                             e_row[:, j * 512:(j + 1) * 512], start=True, stop=True)
            drain(e_full[:, j * 512:(j + 1) * 512], psb[:])

        # ---- T1: per-h transpose of xcp -> xcT1 [w, (h,c)] ----
        for hb in range(8):
            pst = ps_t.tile([128, 512], BF, tag="pst")
            for r in range(8):
                hh = hb * 8 + r
                nc.tensor.transpose(pst[:W, r * 64:(r + 1) * 64],
                                    xcp_r[:, 1 + hh, 1:65],
                                    ident[0:HID, 0:HID])
            drain(xcT1[:, hb * 512:(hb + 1) * 512], pst[:W, :])

        # ---- FFT-W ----
        for j in range(8):
            psf = ps_t.tile([97, 512], DT, tag="pst")
            nc.tensor.matmul(psf[:], fw2, xcT1[:, j * 512:(j + 1) * 512],
                             start=True, stop=True)
            drain(WfP[0:97, j * 512:(j + 1) * 512], psf[:])

        # ---- T2: [97,(h,c)] -> WfRe[h,(c,f)] rows 0:64, WfIm rows 64:128 ----
        WfP_r = WfP[0:97, :].rearrange("p (h c) -> p c h", c=HID)
        for g in range(8):
            pstRI = ps_t.tile([128, 272], BF, tag="pst")
            for r in range(8):
                c = g * 8 + r
                nc.tensor.transpose(pstRI[0:64, r * 34:r * 34 + F],
                                    WfP_r[0:F, c, :], ident[0:F, 0:F])
                nc.tensor.transpose(pstRI[64:128, r * 34:r * 34 + F],
                                    WfP_r[64:97, c, :], ident[64:97, 64:97])
            pv = pstRI.rearrange("p (c t) -> p c t", t=34)[:, :, 0:F]
            ov = tD[:, g * 8 * F:(g + 1) * 8 * F].rearrange("p (c t) -> p c t", t=F)
            drain(ov, pv)

        # ---- B part 2: pooled + proj MLP front ----
        scr_r = scr.rearrange("p (h w) -> p h w", w=W)
        ef_r = e_full.rearrange("p (h w) -> p h w", w=W)
        nc.vector.tensor_tensor(scr_r[:], xcp_r[:, 1:65, 1:65], ef_r[:], OP.mult)
        praw = sv.tile([HID, 1], DT, tag="praw")
        nc.vector.reduce_sum(praw[:], scr[:], AX.X)
        z8b = sv.tile([1, 8], BF, tag="z8b")
        nc.scalar.activation(z8b[:], z8[:], AF.Copy)
        zps = ps_t.tile([HID, 8], DT, tag="pst")
        nc.tensor.matmul(zps[:], onesrow, z8b[:], start=True, stop=True)
        zb = sv.tile([HID, 8], DT, tag="zb")
        nc.vector.tensor_copy(zb[:], zps[:])
        Zv = sv.tile([HID, 1], DT, tag="Zv")
        nc.vector.reduce_sum(Zv[:], zb[:], AX.X)
        zr = sv.tile([HID, 1], DT, tag="zr")
        nc.vector.reciprocal(zr[:], Zv[:])
        pooled = sv.tile([HID, 1], DT, tag="pooled")
        nc.vector.tensor_tensor(pooled[:], praw[:], zr[:], OP.mult)
        psm = ps_t.tile([32, 1], DT, tag="pst")
        nc.tensor.matmul(psm[:], pw1T, pooled[:], start=True, stop=True)
        hv = sv.tile([32, 1], DT, tag="hv")
        nc.scalar.activation(hv[:], psm[:], AF.Identity, bias=pb1)

        # ---- FFT-H forward (packed) + mag cluster for all groups ----
        gd = []
        for g, (c0, nA, nB) in enumerate(GROUPS):
            n = nA * F
            packed = nB > 0
            rows = 128 if packed else 64
            slA = slice(c0 * F, c0 * F + n)
            slB = slice((c0 + nA) * F, (c0 + nA) * F + nB * F)

            psRe = ps_t.tile([128, 512], DT, tag="pst", name=f"psRe{g}")
            psIm = ps_t.tile([128, 512], DT, tag="pst", name=f"psIm{g}")
            nc.tensor.matmul(psRe[0:64, 0:n], chm_lo, WfRe[:, slA],
                             start=True, stop=False)
            nc.tensor.matmul(psRe[0:64, 0:n], shm_hi, WfIm[:, slA],
                             start=False, stop=True)
            nc.tensor.matmul(psIm[0:64, 0:n], chm_hi, WfIm[:, slA],
                             start=True, stop=False)
            nc.tensor.matmul(psIm[0:64, 0:n], nshm_lo, WfRe[:, slA],
                             start=False, stop=True)
            if packed:
                nc.tensor.matmul(psRe[64:128, 0:n], chm_lo, WfRe[:, slB],
                                 start=True, stop=False)
                nc.tensor.matmul(psRe[64:128, 0:n], shm_hi, WfIm[:, slB],
                                 start=False, stop=True)
                nc.tensor.matmul(psIm[64:128, 0:n], chm_hi, WfIm[:, slB],
                                 start=True, stop=False)
                nc.tensor.matmul(psIm[64:128, 0:n], nshm_lo, WfRe[:, slB],
                                 start=False, stop=True)
            ReG = spec.tile([rows, n], DT, tag="sp", name=f"ReG{g}")
            nc.scalar.activation(ReG[:], psRe[0:rows, 0:n], AF.Copy)
            ImG = spec.tile([rows, n], DT, tag="sp", name=f"ImG{g}")
            nc.scalar.activation(ImG[:], psIm[0:rows, 0:n], AF.Identity,
                                 bias=EPS_IM)
            sqR = spec.tile([rows, n], DT, tag="sp", name=f"sqR{g}")
            nc.scalar.activation(sqR[:], ReG[:], AF.Square)
            sqI = spec.tile([rows, n], DT, tag="sp", name=f"sqI{g}")
            nc.scalar.activation(sqI[:], ImG[:], AF.Square)
            m2 = spec.tile([rows, n], DT, tag="sp", name=f"m2{g}")
            nc.vector.tensor_tensor(m2[:], sqR[:], sqI[:], OP.add)
            mag = spec.tile([rows, n], DT, tag="sp", name=f"mag{g}")
            nc.scalar.activation(mag[:], m2[:], AF.Sqrt)
            gd.append((n, rows, ReG, ImG, mag))

        # ---- spectral chains (trig table) + inverse + conv interleave ----
        def conv_chunk(j):
            pc = ps_t.tile([128, 512], DT, tag="pst", name=f"conv{j}")
            for t_ in range(9):
                dy, dx = t_ // 3, t_ % 3
                nc.tensor.matmul(pc[64:128, :], w9t(t_),
                                 xcp_r[:, 8 * j + dy:8 * j + 8 + dy, dx:dx + 64],
                                 start=(t_ == 0), stop=(t_ == 8))
            drain(conv9[:, j * 512:(j + 1) * 512], pc[64:128, :])

        for g, (c0, nA, nB) in enumerate(GROUPS):
            n, rows, ReG, ImG, mag = gd[g]

            def ct(name, dtype=DT):
                return spec.tile([rows, n], dtype, tag="sp",
                                 name=f"{name}{g}")[:]

            den = ct("den")
            nc.vector.tensor_tensor(den, mag[:], ReG[:], OP.add)
            den2 = ct("den2")
            nc.vector.tensor_scalar(den2, den, 1e-30, None, OP.max)
            dri = ct("dri")
            nc.vector.reciprocal(dri, den2)
            q = ct("q")
            nc.vector.tensor_tensor(q, ImG[:], dri, OP.mult)
            aq = ct("aq")
            nc.scalar.activation(aq, q, AF.Arctan)
            vp1 = ct("vp1")
            nc.vector.tensor_tensor(vp1, aq, specc("pw2p", g, n, (0, rows)),
                                    OP.mult)
            vpre = ct("vpre")
            nc.vector.tensor_tensor(vpre, vp1, specc("pb3p", g, n, (0, rows)),
                                    OP.add)
            sinv = ct("sinv")
            nc.scalar.activation(sinv, vpre, AF.Sin)
            k3 = ct("k3")
            nc.vector.tensor_scalar(k3, vpre, PI / 2, 2 * PI, OP.is_gt, OP.mult)
            cos_in = ct("cos_in")
            nc.vector.scalar_tensor_tensor(cos_in, vpre, PI / 2, k3,
                                           OP.add, OP.subtract)
            cosv = ct("cosv")
            nc.scalar.activation(cosv, cos_in, AF.Sin)
            magw = ct("magw")
            nc.vector.tensor_tensor(magw, mag[:], specc("awp", g, n, (0, rows)),
                                    OP.mult)
            mag2 = ct("mag2")
            nc.vector.tensor_tensor(mag2, magw, specc("abp", g, n, (0, rows)),
                                    OP.add)
            Rp = ct("Rp", BF)
            nc.vector.tensor_tensor(Rp, mag2, cosv, OP.mult)
            Ip = ct("Ip", BF)
            nc.vector.tensor_tensor(Ip, mag2, sinv, OP.mult)

            # inverse FFT-H for chunk A (rows 0:64) and chunk B (rows 64:128)
            halves = [(slice(0, 64), c0, nA)]
            if nB > 0:
                halves.append((slice(64, 128), c0 + nA, nB))
            for hs, cc0, nb in halves:
                nn = nb * F
                psR = ps_t.tile([64, 512], DT, tag="pst", name=f"ivR{g}{cc0}")
                lhs_c = chm_lo if hs.start == 0 else chm_hi
                lhs_ns = nshm_lo if hs.start == 0 else nshm_hi
                lhs_s = bfc("shm", (0, 64), 64) if hs.start == 0 else shm_hi
                nc.tensor.matmul(psR[:, 0:nn], lhs_c, Rp[hs, 0:nn],
                                 start=True, stop=False)
                nc.tensor.matmul(psR[:, 0:nn], lhs_ns, Ip[hs, 0:nn],
                                 start=False, stop=True)
                nc.scalar.activation(W2_r[:, cc0:cc0 + nb, 0:F],
                                     psR[:, 0:nn].rearrange("p (c t) -> p c t", t=F),
                                     AF.Copy)
                psI = ps_t.tile([64, 512], DT, tag="pst", name=f"ivI{g}{cc0}")
                nc.tensor.matmul(psI[:, 0:nn], lhs_c, Ip[hs, 0:nn],
                                 start=True, stop=False)
                nc.tensor.matmul(psI[:, 0:nn], lhs_s, Rp[hs, 0:nn],
                                 start=False, stop=True)
                nc.vector.tensor_copy(W2_r[:, cc0:cc0 + nb, F:66],
                                      psI[:, 0:nn].rearrange("p (c t) -> p c t", t=F))
            # interleave conv chunks so PE fills DVE-chain shadows
            for j in range(g * 3, min(g * 3 + 3, 8)):
                conv_chunk(j)

        # ---- W2T + irfft-W -> W3 (rows 64:128) ----
        for g in range(8):
            pst = ps_t.tile([128, 512], BF, tag="pst")
            for r in range(8):
                c = g * 8 + r
                nc.tensor.transpose(pst[0:66, r * 64:(r + 1) * 64],
                                    W2[:, c * 66:(c + 1) * 66], ident[0:H, 0:H])
            drain(W2T[:, g * 512:(g + 1) * 512], pst[0:66, :])
        for j in range(8):
            psw = ps_t.tile([128, 512], DT, tag="pst")
            nc.tensor.matmul(psw[64:128, :], fwi, W2T[:, j * 512:(j + 1) * 512],
                             start=True, stop=True)
            drain(W3[:, j * 512:(j + 1) * 512], psw[64:128, :])

        # ---- T4: W3 [w,(c,h)] -> gx [c,(h,w)] rows 64:128, with |.| ----
        W3_r = W3.rearrange("p (c h) -> p h c", h=H)
        avg8 = sv.tile([128, 8], DT, tag="avg8")
        ident_hi = ident[64:128, 64:128]
        for hb in range(8):
            pst = ps_t.tile([128, 512], BF, tag="pst")
            for r in range(8):
                hh = hb * 8 + r
                nc.tensor.transpose(pst[64:128, r * 64:(r + 1) * 64],
                                    W3_r[:, hh, :], ident_hi)
            nc.scalar.activation(gx[:, hb * 512:(hb + 1) * 512], pst[64:128, :],
                                 AF.Abs, accum_out=avg8[64:128, hb:hb + 1])
        avgn = sv.tile([128, 1], DT, tag="avgn")
        nc.vector.reduce_sum(avgn[64:128, :], avg8[64:128, :], AX.X)
        avgv = sv.tile([128, 1], DT, tag="avgv")
        nc.vector.tensor_scalar(avgv[64:128, :], avgn[64:128, :], 1.0 / HW,
                                None, OP.mult)

        # ---- proj MLP finish (erf gelu) -> cs row + cs col(hi) ----
        e1 = sv.tile([32, 1], DT, tag="e1")
        nc.scalar.activation(e1[:], hv[:], AF.Erf, scale=ERF_SCALE)
        gh = sv.tile([32, 1], DT, tag="gh")
        nc.vector.tensor_scalar(gh[:], e1[:], 1.0, 0.5, OP.add, OP.mult)
        g1v = sv.tile([32, 1], DT, tag="g1v")
        nc.vector.tensor_tensor(g1v[:], hv[:], gh[:], OP.mult)
        psm2 = ps_t.tile([128, 1], DT, tag="pst")
        nc.tensor.matmul(psm2[64:128, :], pw2T, g1v[:], start=True, stop=True)
        csb = sv.tile([128, 1], DT, tag="csb")
        nc.scalar.activation(csb[64:128, :], psm2[64:128, :], AF.Identity,
                             bias=pb2)
        psr = ps_t.tile([1, 64], DT, tag="pst")
        nc.tensor.matmul(psr[:], g1v[:], pw2T, start=True, stop=True)
        csr = sv.tile([1, 64], BF, tag="csr")
        nc.vector.tensor_tensor(csr[:], psr[:], pb2row, OP.add)

        # ---- channel gate MLP (rows 64:128) ----
        psc = ps_t.tile([16, 1], DT, tag="pst")
        nc.tensor.matmul(psc[:], cw1T, avgv[64:128, :], start=True, stop=True)
        chv = sv.tile([16, 1], DT, tag="chv")
        nc.scalar.activation(chv[:], psc[:], AF.Identity, bias=cb1)
        ce1 = sv.tile([16, 1], DT, tag="ce1")
        nc.scalar.activation(ce1[:], chv[:], AF.Erf, scale=ERF_SCALE)
        cgh = sv.tile([16, 1], DT, tag="cgh")
        nc.vector.tensor_scalar(cgh[:], ce1[:], 1.0, 0.5, OP.add, OP.mult)
        cg1 = sv.tile([16, 1], DT, tag="cg1")
        nc.vector.tensor_tensor(cg1[:], chv[:], cgh[:], OP.mult)
        psc2 = ps_t.tile([128, 1], DT, tag="pst")
        nc.tensor.matmul(psc2[64:128, :], cw2T, cg1[:], start=True, stop=True)
        cgb = sv.tile([128, 1], DT, tag="cgb")
        nc.scalar.activation(cgb[64:128, :], psc2[64:128, :], AF.Sigmoid,
                             bias=cb2)
        cscg = sv.tile([128, 1], DT, tag="cscg")
        nc.vector.tensor_tensor(cscg[64:128, :], csb[64:128, :], cgb[64:128, :],
                                OP.mult)
        bstar = sv.tile([128, 1], DT, tag="bstar")
        nc.vector.tensor_tensor(bstar[64:128, :], cscg[64:128, :], cbvec,
                                OP.mult)

        # ---- spatial gate + y assembly (all rows 64:128) ----
        for j in range(8):
            ps1 = ps_t.tile([1, 512], DT, tag="pst")
            nc.tensor.matmul(ps1[:], siw, conv9[:, j * 512:(j + 1) * 512],
                             start=True, stop=True)
            nc.scalar.activation(sgrow[:, j * 512:(j + 1) * 512], ps1[:],
                                 AF.Sigmoid, bias=scalars["si_b_eff"])
            psb = ps_t.tile([128, 512], DT, tag="pst")
            nc.tensor.matmul(psb[64:128, :], csr[:],
                             sgrow[:, j * 512:(j + 1) * 512], start=True, stop=True)
            y1 = spec.tile([128, 512], BF, tag="sp", name=f"y1_{j}")
            nc.vector.tensor_tensor(y1[64:128, :], gx[:, j * 512:(j + 1) * 512],
                                    psb[64:128, :], OP.mult)
            tl = spec.tile([128, 512], BF, tag="sp", name=f"tl_{j}")
            nc.scalar.activation(tl[64:128, :], conv9[:, j * 512:(j + 1) * 512],
                                 AF.Identity, scale=cscg[64:128, :],
                                 bias=bstar[64:128, :])
            nc.vector.tensor_tensor(ypb[:, j * 512:(j + 1) * 512],
                                    y1[64:128, :], tl[64:128, :], OP.add)

        # ---- tail: out[hw, D] = ypb^T @ tailT, bf16 out in 4-chunk DMAs ----
        out_r = out.rearrange("(g i p) d -> g p i d", i=4, p=128)
        for gq in range(8):
            osb = outp.tile([128, 4 * D], BF, tag="osb")
            for i4 in range(4):
                i = gq * 4 + i4
                pst_ = ps_tail.tile([128, D], DT, tag="tacc")
                nc.tensor.matmul(pst_[:, 0:512], ypb[:, i * 128:(i + 1) * 128],
                                 tailT[:, 0:512], start=True, stop=True)
                nc.tensor.matmul(pst_[:, 512:D], ypb[:, i * 128:(i + 1) * 128],
                                 tailT[:, 512:D], start=True, stop=True)
                drain(osb[:, i4 * D:(i4 + 1) * D], pst_[:])
            nc.sync.dma_start(out=out_r[gq], in_=osb.rearrange("p (i d) -> p i d", d=D))

    if compile:
        nc.compile()
    return nc


def host_prep(inp):
    p = {k: np.ascontiguousarray(np.asarray(v, np.float32)) for k, v in inp.items()}
    s = p["bn_w"] / np.sqrt(p["bn_var"] + 1e-5)
    W1 = (p["head_w"] * s[:, None]).astype(np.float64)
    b1 = (p["head_b"] - p["bn_mean"]) * s + p["bn_b"]
    w = np.arange(W)
    f = np.arange(F)
    h = np.arange(H)
    Cw = np.cos(2 * np.pi * np.outer(w, f) / W) / 8.0
    Sw = -np.sin(2 * np.pi * np.outer(w, f) / W) / 8.0
    Fw2 = np.concatenate([Cw, np.zeros((W, 31)), Sw], axis=1)      # [64, 97]
    Ch = np.cos(2 * np.pi * np.outer(h, h) / H) / 8.0
    Sh = np.sin(2 * np.pi * np.outer(h, h) / H) / 8.0
    cf = np.full(F, 2.0)
    cf[0] = 1.0
    cf[F - 1] = 1.0
    Fwi = np.concatenate([
        (cf[:, None] * np.cos(2 * np.pi * np.outer(f, w) / W)) / 8.0,
        -(cf[:, None] * np.sin(2 * np.pi * np.outer(f, w) / W)) / 8.0,
    ], axis=0)                                                     # [66, 64]
    w9 = (p["conv_A"] @ p["conv_B"])                               # [64, 9]

    cf32 = np.zeros((128, F32_W), np.float32)

    def put32(name, rows, arr):
        arr = np.asarray(arr, np.float32)
        if arr.ndim == 1:
            arr = arr[:, None]
        cf32[rows[0]:rows[0] + arr.shape[0],
             F32_COLS[name]:F32_COLS[name] + arr.shape[1]] = arr

    put32("b1", (0,), b1)
    put32("pb1", (0,), p["proj_b1"])
    put32("cb1", (0,), p["ci_b1"])
    put32("cb2", (64,), p["ci_b2"])
    put32("pb2", (64,), p["proj_b2"])
    put32("cbvec", (64,), p["conv_bias"])
    put32("pw1T", (0,), np.ascontiguousarray(p["proj_w1"].T))
    put32("pw2T", (0,), np.ascontiguousarray(p["proj_w2"].T))
    put32("cw1T", (64,), np.ascontiguousarray(p["ci_w1"].T))
    put32("cw2T", (0,), np.ascontiguousarray(p["ci_w2"].T))
    put32("pb2row", (0,), p["proj_b2"][None, :])

    cbf = np.zeros((128, BF_W), np.float64)

    def putbf(name, rows, arr):
        arr = np.asarray(arr, np.float64)
        if arr.ndim == 1:
            arr = arr[:, None]
        cbf[rows[0]:rows[0] + arr.shape[0],
            BF_COLS[name]:BF_COLS[name] + arr.shape[1]] = arr

    putbf("w1t", (0,), np.ascontiguousarray(W1.T).reshape(NK, 128, HID)
          .transpose(1, 0, 2).reshape(128, NK * HID))
    putbf("fw2", (0,), Fw2)
    putbf("chm", (0,), Ch)
    putbf("chm", (64,), Ch)
    putbf("shm", (0,), Sh)
    putbf("shm", (64,), Sh)
    putbf("nshm", (0,), -Sh)
    putbf("nshm", (64,), -Sh)
    putbf("fwi", (0,), Fwi)
    putbf("ident", (0,), np.eye(128))
    w9d = np.zeros((64, 9 * 64))
    for t_ in range(9):
        w9d[:, t_ * 64:(t_ + 1) * 64] = np.diag(w9[:, t_])
    putbf("w9d", (0,), w9d)
    putbf("tailT", (64,), np.ascontiguousarray(p["tail_w"].T))
    putbf("onesrow", (0,), np.ones((1, 64)))
    putbf("cwv", (0,), np.ascontiguousarray(p["compress_w"].T))
    putbf("siw", (64,), np.ascontiguousarray(p["si_w"].T))

    def spack(name, vec):
        buf = np.zeros((128, NSP))
        for g, (c0, nA, nB) in enumerate(GROUPS):
            base = GBASE[g]
            buf[0:64, base:base + nA * F] = np.repeat(vec[c0:c0 + nA], F)[None, :]
            if nB > 0:
                buf[64:128, base:base + nB * F] = \
                    np.repeat(vec[c0 + nA:c0 + nA + nB], F)[None, :]
        putbf(name, (0,), buf)

    spack("awp", p["amp_w"])
    spack("abp", p["amp_b"])
    spack("pw2p", 2.0 * p["pha_w"])
    spack("pb3p", p["pha_b"])

    consts = dict(
        cf32=np.ascontiguousarray(cf32),
        cbf=np.ascontiguousarray(cbf.astype(ml_dtypes.bfloat16)),
    )
    scalars = dict(
        compress_b=float(p["compress_b"][0]),
        si_b_eff=float(p["si_b"][0] + p["si_w"][0] @ p["conv_bias"]),
    )
    return consts, scalars


def kernel(**inputs):
    x = np.asarray(inputs["x"], np.float32)
    N = x.shape[0]
    consts, scalars = host_prep(inputs)
    nc = build_graph(scalars)
    in_maps = []
    for i in range(N):
        xu = x[i].reshape(HW, D).view(np.uint32)
        xr = ((xu + 0x7FFF + ((xu >> 16) & 1)) >> 16).astype(np.uint16)  # bf16 RNE
        m = {"x": xr.view(ml_dtypes.bfloat16)}
        m.update(consts)
        in_maps.append(m)
    res = run_bass_kernel_spmd(nc, in_maps, core_ids=list(range(N)))
    outs = np.stack([np.asarray(res.results[i]["out"], np.float32).reshape(H, W, D)
                     for i in range(N)])
    return outs.astype(np.float32)


# revision 6
# speedup vs baseline: 1.0065x; 1.0020x over previous
"""AdaptIR forward as a Bass/Tile kernel on 8 TRN2 NeuronCores.

Data-parallel over batch N=8: one sample per core, all params replicated.
Self-contained: hardcodes all shapes; no sibling imports.

v2 layout: spectral pointwise packed 2 c-chunks onto 128 partitions,
depthwise conv on PE (diag stationaries), consolidated const DMAs,
half-angle atan2, bf16 output.

Partition placement convention (engines are lane-locked):
  rows 0:64  : xcp(padded head out), e_full/scr, WfRe, xcT1, W2, chunkA spectral
  rows 64:128: conv9, gx, ypb, WfIm, W3, tailT/siw consts, chunkB spectral
"""
import math
from contextlib import ExitStack

import ml_dtypes
import numpy as np

import concourse.bacc as bacc
import concourse.bass as bass
import concourse.mybir as mybir
import concourse.tile as tile
from concourse.bass_utils import run_bass_kernel_spmd

DT = mybir.dt.float32
BF = mybir.dt.bfloat16
AF = mybir.ActivationFunctionType
OP = mybir.AluOpType
AX = mybir.AxisListType

D, HID, F = 896, 64, 33
H = W = 64
HW = H * W              # 4096
NK = D // 128           # 7
NS = HID * F            # 2112
PI = math.pi
ERF_SCALE = 0.7071067811865476
EPS_IM = 1e-12

# spectral groups: (c0, nbA, nbB) -> chunk A rows 0:64, chunk B rows 64:128
GROUPS = [(0, 13, 13), (26, 13, 13), (52, 6, 6)]
GBASE = [0, 429, 858]
NSP = 1188


def _cols(widths):
    off, c = {}, 0
    for name, wd in widths:
        off[name] = c
        c += wd
    return off, c


F32_COLS, F32_W = _cols([
    ("b1", 1), ("pb1", 1), ("cb1", 1), ("cb2", 1), ("pb2", 1), ("cbvec", 1),
    ("pw1T", 32), ("pw2T", 64), ("cw1T", 16), ("cw2T", 64), ("pb2row", 64),
])
BF_COLS, BF_W = _cols([
    ("w1t", 448), ("fw2", 97), ("chm", 64), ("shm", 64), ("nshm", 64),
    ("fwi", 64), ("ident", 128), ("w9d", 576), ("tailT", 896),
    ("onesrow", 64), ("cwv", 1), ("siw", 1),
    ("awp", NSP), ("abp", NSP), ("pw2p", NSP), ("pb3p", NSP),
])


def build_graph(scalars, compile=True, trace_sim=False):
    nc = bacc.Bacc()
    x = nc.declare_dram_parameter("x", [HW, D], BF, isOutput=False)
    out = nc.declare_dram_parameter("out", [HW, D], BF, isOutput=True)
    cf32_d = nc.declare_dram_parameter("cf32", [128, F32_W], DT, isOutput=False)
    cbf_d = nc.declare_dram_parameter("cbf", [128, BF_W], BF, isOutput=False)

    for i, v in enumerate(sorted({scalars["compress_b"], scalars["si_b_eff"],
                                  EPS_IM, ERF_SCALE})):
        t = nc.alloc_sbuf_tensor(f"constap-{i}", [128, 1], DT)
        nc.gpsimd.memset(t.ap(), v)
        nc.const_aps.aps[(DT, v)] = t.ap()
    nc.all_engine_barrier()

    with tile.TileContext(nc, trace_sim=trace_sim) as tc, ExitStack() as ctx:
        cpool = ctx.enter_context(tc.tile_pool(name="consts", bufs=1))
        ps_t = ctx.enter_context(tc.tile_pool(name="ps_t", bufs=4, space="PSUM"))
        ps_tail = ctx.enter_context(tc.tile_pool(name="ps_tail", bufs=2, space="PSUM"))
        xTp = ctx.enter_context(tc.tile_pool(name="xT", bufs=10))
        persist = ctx.enter_context(tc.tile_pool(name="persist", bufs=1))
        spec = ctx.enter_context(tc.tile_pool(name="spec", bufs=16))
        outp = ctx.enter_context(tc.tile_pool(name="outp", bufs=2))
        sv = ctx.enter_context(tc.tile_pool(name="sv", bufs=1))

        cf32 = cpool.tile([128, F32_W], DT, tag="cf32")
        nc.sync.dma_start(out=cf32[:], in_=cf32_d[:])
        cbf = cpool.tile([128, BF_W], BF, tag="cbf")
        nc.sync.dma_start(out=cbf[:], in_=cbf_d[:])

        def f32c(name, rows, wd=1):
            c0 = F32_COLS[name]
            return cf32[rows[0]:rows[1], c0:c0 + wd]

        def bfc(name, rows, wd=1):
            c0 = BF_COLS[name]
            return cbf[rows[0]:rows[1], c0:c0 + wd]

        b1 = f32c("b1", (0, HID))
        pb1 = f32c("pb1", (0, 32))
        cb1 = f32c("cb1", (0, 16))
        cb2 = f32c("cb2", (64, 128))
        pb2 = f32c("pb2", (64, 128))
        cbvec = f32c("cbvec", (64, 128))
        pw1T = f32c("pw1T", (0, HID), 32)
        pw2T = f32c("pw2T", (0, 32), 64)
        cw1T = f32c("cw1T", (64, 128), 16)
        cw2T = f32c("cw2T", (0, 16), 64)
        pb2row = f32c("pb2row", (0, 1), 64)

        fw2 = bfc("fw2", (0, W), 97)
        chm_lo = bfc("chm", (0, 64), 64)
        shm_lo = bfc("shm", (0, 64), 64)
        shm_hi = bfc("shm", (64, 128), 64)
        chm_hi = bfc("chm", (64, 128), 64)
        nshm_lo = bfc("nshm", (0, 64), 64)
        nshm_hi = bfc("nshm", (64, 128), 64)
        fwi = bfc("fwi", (0, 66), 64)
        ident = bfc("ident", (0, 128), 128)
        tailT = bfc("tailT", (64, 128), D)
        onesrow = bfc("onesrow", (0, 1), 64)
        cwv = bfc("cwv", (0, HID))
        siw = bfc("siw", (64, 128))

        def w1tk(k):
            c0 = BF_COLS["w1t"] + 64 * k
            return cbf[:, c0:c0 + 64]

        def w9t(t_):
            c0 = BF_COLS["w9d"] + 64 * t_
            return cbf[0:HID, c0:c0 + 64]

        def specc(name, g, n, rows=(0, 128)):
            c0 = BF_COLS[name] + GBASE[g]
            return cbf[rows[0]:rows[1], c0:c0 + n]

        # ---- persistent SBUF tiles (64-row tensors packed in pairs) ----
        tA = persist.tile([128, 66 * 66], BF, tag="tA")   # xcp | conv9
        xcp = tA[0:HID, :]
        xcp_r = xcp.rearrange("p (h w) -> p h w", w=66)
        conv9 = tA[64:128, 0:HW]
        tB = persist.tile([128, HW], BF, tag="tB")        # scr | gx
        scr = tB[0:HID, :]
        gx = tB[64:128, :]
        tC = persist.tile([128, HW], BF, tag="tC")        # e_full | ypb
        e_full = tC[0:HID, :]
        ypb = tC[64:128, :]
        tD = persist.tile([HID, NS], BF, tag="tD")        # WfRe
        WfRe = tD[0:HID, :]
        tDi = persist.tile([HID, NS], BF, tag="tDi")      # WfIm (base 0!)
        WfIm = tDi[0:HID, :]
        tE = persist.tile([128, HW], BF, tag="tE")        # xcT1 | W3
        xcT1 = tE[0:HID, :]
        W3 = tE[64:128, :]
        WfP = persist.tile([128, HW], BF, tag="WfP")      # rows 0:97 used
        W2 = persist.tile([HID, HID * 66], BF, tag="W2")
        W2_r = W2.rearrange("p (c t) -> p c t", t=66)
        W2T = persist.tile([66, HW], BF, tag="W2T")
        e_row = persist.tile([1, HW], BF, tag="e_row")
        sgrow = persist.tile([1, HW], BF, tag="sgrow")

        drain_flip = [0]

        def drain(dst, src, bias=None):
            if drain_flip[0] % 2 == 0:
                if bias is None:
                    nc.scalar.activation(dst, src, AF.Copy)
                else:
                    nc.scalar.activation(dst, src, AF.Identity, bias=bias)
            else:
                if bias is None:
                    nc.vector.tensor_copy(dst, src)
                else:
                    nc.vector.tensor_scalar(dst, src, bias, None, OP.add)
            drain_flip[0] += 1

        # ---- zero the 1-px border of the padded conv buffer ----
        nc.gpsimd.memset(xcp_r[:, 0, :], 0.0)
        nc.gpsimd.memset(xcp_r[:, 65, :], 0.0)
        nc.gpsimd.memset(xcp_r[:, 1:65, 0:1], 0.0)
        nc.gpsimd.memset(xcp_r[:, 1:65, 65:66], 0.0)

        # ---- x loads: 14 transpose DMAs of [2048, 128] ----
        xt = {}
        for jh in range(2):
            for k in range(NK):
                t = xTp.tile([128, 2048], BF, tag="xt", name=f"xt_{jh}_{k}")
                nc.sync.dma_start(
                    out=t[:],
                    in_=x[jh * 2048:(jh + 1) * 2048, k * 128:(k + 1) * 128],
                    transpose=True)
                xt[(jh, k)] = t

        # ---- A: head matmul, drain straight into padded xcp ----
        for j in range(8):
            jh, jl = j // 4, j % 4
            ps_h = ps_t.tile([HID, 512], DT, tag="pst")
            for k in range(NK):
                nc.tensor.matmul(ps_h[:], w1tk(k),
                                 xt[(jh, k)][:, jl * 512:(jl + 1) * 512],
                                 start=(k == 0), stop=(k == NK - 1))
            drain(xcp_r[:, 1 + 8 * j:9 + 8 * j, 1:65], ps_h[:], bias=b1)

        # ---- B part 1: compress -> exp (+Z accum) -> e broadcast ----
        z8 = sv.tile([1, 8], DT, tag="z8")
        for j in range(8):
            ps1 = ps_t.tile([1, 512], DT, tag="pst")
            nc.tensor.matmul(ps1[:], cwv,
                             xcp_r[:, 1 + 8 * j:9 + 8 * j, 1:65],
                             start=True, stop=True)
            nc.scalar.activation(e_row[:, j * 512:(j + 1) * 512], ps1[:],
                                 AF.Exp, bias=scalars["compress_b"],
                                 accum_out=z8[:, j:j + 1])
            psb = ps_t.tile([HID, 512], DT, tag="pst")
            nc.tensor.matmul(psb[:], onesrow,
                             e_row[:, j * 512:(j + 1) * 512], start=True, stop=True)
            drain(e_full[:, j * 512:(j + 1) * 512], psb[:])

        # ---- T1: per-h transpose of xcp -> xcT1 [w, (h,c)] ----
        for hb in range(8):
            pst = ps_t.tile([128, 512], BF, tag="pst")
            for r in range(8):
                hh = hb * 8 + r
                nc.tensor.transpose(pst[:W, r * 64:(r + 1) * 64],
                                    xcp_r[:, 1 + hh, 1:65],
                                    ident[0:HID, 0:HID])
            drain(xcT1[:, hb * 512:(hb + 1) * 512], pst[:W, :])

        # ---- FFT-W ----
        for j in range(8):
            psf = ps_t.tile([97, 512], DT, tag="pst")
            nc.tensor.matmul(psf[:], fw2, xcT1[:, j * 512:(j + 1) * 512],
                             start=True, stop=True)
            drain(WfP[0:97, j * 512:(j + 1) * 512], psf[:])

        # ---- T2: [97,(h,c)] -> WfRe[h,(c,f)] rows 0:64, WfIm rows 64:128 ----
        WfP_r = WfP[0:97, :].rearrange("p (h c) -> p c h", c=HID)
        for g in range(8):
            pstRI = ps_t.tile([128, 272], BF, tag="pst")
            for r in range(8):
                c = g * 8 + r
                nc.tensor.transpose(pstRI[0:64, r * 34:r * 34 + F],
                                    WfP_r[0:F, c, :], ident[0:F, 0:F])
                nc.tensor.transpose(pstRI[64:128, r * 34:r * 34 + F],
                                    WfP_r[64:97, c, :], ident[64:97, 64:97])
            pv = pstRI.rearrange("p (c t) -> p c t", t=34)[:, :, 0:F]
            ov = tD[:, g * 8 * F:(g + 1) * 8 * F].rearrange("p (c t) -> p c t", t=F)
            drain(ov, pv)

        # ---- B part 2: pooled + proj MLP front ----
        scr_r = scr.rearrange("p (h w) -> p h w", w=W)
        ef_r = e_full.rearrange("p (h w) -> p h w", w=W)
        nc.vector.tensor_tensor(scr_r[:], xcp_r[:, 1:65, 1:65], ef_r[:], OP.mult)
        praw = sv.tile([HID, 1], DT, tag="praw")
        nc.vector.reduce_sum(praw[:], scr[:], AX.X)
        z8b = sv.tile([1, 8], BF, tag="z8b")
        nc.scalar.activation(z8b[:], z8[:], AF.Copy)
        zps = ps_t.tile([HID, 8], DT, tag="pst")
        nc.tensor.matmul(zps[:], onesrow, z8b[:], start=True, stop=True)
        zb = sv.tile([HID, 8], DT, tag="zb")
        nc.vector.tensor_copy(zb[:], zps[:])
        Zv = sv.tile([HID, 1], DT, tag="Zv")
        nc.vector.reduce_sum(Zv[:], zb[:], AX.X)
        zr = sv.tile([HID, 1], DT, tag="zr")
        nc.vector.reciprocal(zr[:], Zv[:])
        pooled = sv.tile([HID, 1], DT, tag="pooled")
        nc.vector.tensor_tensor(pooled[:], praw[:], zr[:], OP.mult)
        psm = ps_t.tile([32, 1], DT, tag="pst")
        nc.tensor.matmul(psm[:], pw1T, pooled[:], start=True, stop=True)
        hv = sv.tile([32, 1], DT, tag="hv")
        nc.scalar.activation(hv[:], psm[:], AF.Identity, bias=pb1)

        # ---- FFT-H forward (packed) + mag cluster for all groups ----
        gd = []
        for g, (c0, nA, nB) in enumerate(GROUPS):
            n = nA * F
            packed = nB > 0
            rows = 128 if packed else 64
            slA = slice(c0 * F, c0 * F + n)
            slB = slice((c0 + nA) * F, (c0 + nA) * F + nB * F)

            psRe = ps_t.tile([128, 512], DT, tag="pst", name=f"psRe{g}")
            psIm = ps_t.tile([128, 512], DT, tag="pst", name=f"psIm{g}")
            nc.tensor.matmul(psRe[0:64, 0:n], chm_lo, WfRe[:, slA],
                             start=True, stop=False)
            nc.tensor.matmul(psRe[0:64, 0:n], shm_hi, WfIm[:, slA],
                             start=False, stop=True)
            nc.tensor.matmul(psIm[0:64, 0:n], chm_hi, WfIm[:, slA],
                             start=True, stop=False)
            nc.tensor.matmul(psIm[0:64, 0:n], nshm_lo, WfRe[:, slA],
                             start=False, stop=True)
            if packed:
                nc.tensor.matmul(psRe[64:128, 0:n], chm_lo, WfRe[:, slB],
                                 start=True, stop=False)
                nc.tensor.matmul(psRe[64:128, 0:n], shm_hi, WfIm[:, slB],
                                 start=False, stop=True)
                nc.tensor.matmul(psIm[64:128, 0:n], chm_hi, WfIm[:, slB],
                                 start=True, stop=False)
                nc.tensor.matmul(psIm[64:128, 0:n], nshm_lo, WfRe[:, slB],
                                 start=False, stop=True)
            ReG = spec.tile([rows, n], DT, tag="sp", name=f"ReG{g}")
            nc.scalar.activation(ReG[:], psRe[0:rows, 0:n], AF.Copy)
            ImG = spec.tile([rows, n], DT, tag="sp", name=f"ImG{g}")
            nc.scalar.activation(ImG[:], psIm[0:rows, 0:n], AF.Identity,
                                 bias=EPS_IM)
            sqR = spec.tile([rows, n], DT, tag="sp", name=f"sqR{g}")
            nc.scalar.activation(sqR[:], ReG[:], AF.Square)
            sqI = spec.tile([rows, n], DT, tag="sp", name=f"sqI{g}")
            nc.scalar.activation(sqI[:], ImG[:], AF.Square)
            m2 = spec.tile([rows, n], DT, tag="sp", name=f"m2{g}")
            nc.vector.tensor_tensor(m2[:], sqR[:], sqI[:], OP.add)
            mag = spec.tile([rows, n], DT, tag="sp", name=f"mag{g}")
            nc.scalar.activation(mag[:], m2[:], AF.Sqrt)
            gd.append((n, rows, ReG, ImG, mag))

        # ---- spectral chains (trig table) + inverse + conv interleave ----
        def conv_chunk(j):
            pc = ps_t.tile([128, 512], DT, tag="pst", name=f"conv{j}")
            for t_ in range(9):
                dy, dx = t_ // 3, t_ % 3
                nc.tensor.matmul(pc[64:128, :], w9t(t_),
                                 xcp_r[:, 8 * j + dy:8 * j + 8 + dy, dx:dx + 64],
                                 start=(t_ == 0), stop=(t_ == 8))
            drain(conv9[:, j * 512:(j + 1) * 512], pc[64:128, :])

        for g, (c0, nA, nB) in enumerate(GROUPS):
            n, rows, ReG, ImG, mag = gd[g]

            def ct(name, dtype=DT):
                return spec.tile([rows, n], dtype, tag="sp",
                                 name=f"{name}{g}")[:]

            den = ct("den")
            nc.vector.tensor_tensor(den, mag[:], ReG[:], OP.add)
            den2 = ct("den2")
            nc.vector.tensor_scalar(den2, den, 1e-30, None, OP.max)
            dri = ct("dri")
            nc.vector.reciprocal(dri, den2)
            q = ct("q")
            nc.vector.tensor_tensor(q, ImG[:], dri, OP.mult)
            aq = ct("aq")
            nc.scalar.activation(aq, q, AF.Arctan)
            vp1 = ct("vp1")
            nc.vector.tensor_tensor(vp1, aq, specc("pw2p", g, n, (0, rows)),
                                    OP.mult)
            vpre = ct("vpre")
            nc.vector.tensor_tensor(vpre, vp1, specc("pb3p", g, n, (0, rows)),
                                    OP.add)
            sinv = ct("sinv")
            nc.scalar.activation(sinv, vpre, AF.Sin)
            k3 = ct("k3")
            nc.vector.tensor_scalar(k3, vpre, PI / 2, 2 * PI, OP.is_gt, OP.mult)
            cos_in = ct("cos_in")
            nc.vector.scalar_tensor_tensor(cos_in, vpre, PI / 2, k3,
                                           OP.add, OP.subtract)
            cosv = ct("cosv")
            nc.scalar.activation(cosv, cos_in, AF.Sin)
            magw = ct("magw")
            nc.vector.tensor_tensor(magw, mag[:], specc("awp", g, n, (0, rows)),
                                    OP.mult)
            mag2 = ct("mag2")
            nc.vector.tensor_tensor(mag2, magw, specc("abp", g, n, (0, rows)),
                                    OP.add)
            Rp = ct("Rp", BF)
            nc.vector.tensor_tensor(Rp, mag2, cosv, OP.mult)
            Ip = ct("Ip", BF)
            nc.vector.tensor_tensor(Ip, mag2, sinv, OP.mult)

            # inverse FFT-H for chunk A (rows 0:64) and chunk B (rows 64:128)
            halves = [(slice(0, 64), c0, nA)]
            if nB > 0:
                halves.append((slice(64, 128), c0 + nA, nB))
            for hs, cc0, nb in halves:
                nn = nb * F
                psR = ps_t.tile([64, 512], DT, tag="pst", name=f"ivR{g}{cc0}")
                lhs_c = chm_lo if hs.start == 0 else chm_hi
                lhs_ns = nshm_lo if hs.start == 0 else nshm_hi
                lhs_s = bfc("shm", (0, 64), 64) if hs.start == 0 else shm_hi
                nc.tensor.matmul(psR[:, 0:nn], lhs_c, Rp[hs, 0:nn],
                                 start=True, stop=False)
                nc.tensor.matmul(psR[:, 0:nn], lhs_ns, Ip[hs, 0:nn],
                                 start=False, stop=True)
                nc.scalar.activation(W2_r[:, cc0:cc0 + nb, 0:F],
                                     psR[:, 0:nn].rearrange("p (c t) -> p c t", t=F),
                                     AF.Copy)
                psI = ps_t.tile([64, 512], DT, tag="pst", name=f"ivI{g}{cc0}")
                nc.tensor.matmul(psI[:, 0:nn], lhs_c, Ip[hs, 0:nn],
                                 start=True, stop=False)
                nc.tensor.matmul(psI[:, 0:nn], lhs_s, Rp[hs, 0:nn],
                                 start=False, stop=True)
                nc.vector.tensor_copy(W2_r[:, cc0:cc0 + nb, F:66],
                                      psI[:, 0:nn].rearrange("p (c t) -> p c t", t=F))
            # interleave conv chunks so PE fills DVE-chain shadows
            for j in range(g * 3, min(g * 3 + 3, 8)):
                conv_chunk(j)

        # ---- W2T + irfft-W -> W3 (rows 64:128) ----
        for g in range(8):
            pst = ps_t.tile([128, 512], BF, tag="pst")
            for r in range(8):
                c = g * 8 + r
                nc.tensor.transpose(pst[0:66, r * 64:(r + 1) * 64],
                                    W2[:, c * 66:(c + 1) * 66], ident[0:H, 0:H])
            drain(W2T[:, g * 512:(g + 1) * 512], pst[0:66, :])
        for j in range(8):
            psw = ps_t.tile([128, 512], DT, tag="pst")
            nc.tensor.matmul(psw[64:128, :], fwi, W2T[:, j * 512:(j + 1) * 512],
                             start=True, stop=True)
            drain(W3[:, j * 512:(j + 1) * 512], psw[64:128, :])

        # ---- T4: W3 [w,(c,h)] -> gx [c,(h,w)] rows 64:128, with |.| ----
        W3_r = W3.rearrange("p (c h) -> p h c", h=H)
        avg8 = sv.tile([128, 8], DT, tag="avg8")
        ident_hi = ident[64:128, 64:128]
        for hb in range(8):
            pst = ps_t.tile([128, 512], BF, tag="pst")
            for r in range(8):
                hh = hb * 8 + r
                nc.tensor.transpose(pst[64:128, r * 64:(r + 1) * 64],
                                    W3_r[:, hh, :], ident_hi)
            nc.scalar.activation(gx[:, hb * 512:(hb + 1) * 512], pst[64:128, :],
                                 AF.Abs, accum_out=avg8[64:128, hb:hb + 1])
        avgn = sv.tile([128, 1], DT, tag="avgn")
        nc.vector.reduce_sum(avgn[64:128, :], avg8[64:128, :], AX.X)
        avgv = sv.tile([128, 1], DT, tag="avgv")
        nc.vector.tensor_scalar(avgv[64:128, :], avgn[64:128, :], 1.0 / HW,
                                None, OP.mult)

        # ---- proj MLP finish (erf gelu) -> cs row + cs col(hi) ----
        e1 = sv.tile([32, 1], DT, tag="e1")
        nc.scalar.activation(e1[:], hv[:], AF.Erf, scale=ERF_SCALE)
        gh = sv.tile([32, 1], DT, tag="gh")
        nc.vector.tensor_scalar(gh[:], e1[:], 1.0, 0.5, OP.add, OP.mult)
        g1v = sv.tile([32, 1], DT, tag="g1v")
        nc.vector.tensor_tensor(g1v[:], hv[:], gh[:], OP.mult)
        psm2 = ps_t.tile([128, 1], DT, tag="pst")
        nc.tensor.matmul(psm2[64:128, :], pw2T, g1v[:], start=True, stop=True)
        csb = sv.tile([128, 1], DT, tag="csb")
        nc.scalar.activation(csb[64:128, :], psm2[64:128, :], AF.Identity,
                             bias=pb2)
        psr = ps_t.tile([1, 64], DT, tag="pst")
        nc.tensor.matmul(psr[:], g1v[:], pw2T, start=True, stop=True)
        csr = sv.tile([1, 64], BF, tag="csr")
        nc.vector.tensor_tensor(csr[:], psr[:], pb2row, OP.add)

        # ---- channel gate MLP (rows 64:128) ----
        psc = ps_t.tile([16, 1], DT, tag="pst")
        nc.tensor.matmul(psc[:], cw1T, avgv[64:128, :], start=True, stop=True)
        chv = sv.tile([16, 1], DT, tag="chv")
        nc.scalar.activation(chv[:], psc[:], AF.Identity, bias=cb1)
        ce1 = sv.tile([16, 1], DT, tag="ce1")
        nc.scalar.activation(ce1[:], chv[:], AF.Erf, scale=ERF_SCALE)
        cgh = sv.tile([16, 1], DT, tag="cgh")
        nc.vector.tensor_scalar(cgh[:], ce1[:], 1.0, 0.5, OP.add, OP.mult)
        cg1 = sv.tile([16, 1], DT, tag="cg1")
        nc.vector.tensor_tensor(cg1[:], chv[:], cgh[:], OP.mult)
        psc2 = ps_t.tile([128, 1], DT, tag="pst")
        nc.tensor.matmul(psc2[64:128, :], cw2T, cg1[:], start=True, stop=True)
        cgb = sv.tile([128, 1], DT, tag="cgb")
        nc.scalar.activation(cgb[64:128, :], psc2[64:128, :], AF.Sigmoid,
                             bias=cb2)
        cscg = sv.tile([128, 1], DT, tag="cscg")
        nc.vector.tensor_tensor(cscg[64:128, :], csb[64:128, :], cgb[64:128, :],
                                OP.mult)
        bstar = sv.tile([128, 1], DT, tag="bstar")
        nc.vector.tensor_tensor(bstar[64:128, :], cscg[64:128, :], cbvec,
                                OP.mult)

        # ---- spatial gate + y assembly (all rows 64:128) ----
        for j in range(8):
            ps1 = ps_t.tile([1, 512], DT, tag="pst")
            nc.tensor.matmul(ps1[:], siw, conv9[:, j * 512:(j + 1) * 512],
                             start=True, stop=True)
            nc.scalar.activation(sgrow[:, j * 512:(j + 1) * 512], ps1[:],
                                 AF.Sigmoid, bias=scalars["si_b_eff"])
            psb = ps_t.tile([128, 512], DT, tag="pst")
            nc.tensor.matmul(psb[64:128, :], csr[:],
                             sgrow[:, j * 512:(j + 1) * 512], start=True, stop=True)
            y1 = spec.tile([128, 512], BF, tag="sp", name=f"y1_{j}")
            nc.vector.tensor_tensor(y1[64:128, :], gx[:, j * 512:(j + 1) * 512],
                                    psb[64:128, :], OP.mult)
            tl = spec.tile([128, 512], BF, tag="sp", name=f"tl_{j}")
            nc.scalar.activation(tl[64:128, :], conv9[:, j * 512:(j + 1) * 512],
                                 AF.Identity, scale=cscg[64:128, :],
                                 bias=bstar[64:128, :])
            nc.vector.tensor_tensor(ypb[:, j * 512:(j + 1) * 512],
                                    y1[64:128, :], tl[64:128, :], OP.add)

        # ---- tail: out[hw, D] = ypb^T @ tailT, bf16 out in 4-chunk DMAs ----
        out_r = out.rearrange("(g i p) d -> g p i d", i=4, p=128)
        for gq in range(8):
            osb = outp.tile([128, 4 * D], BF, tag="osb")
            for i4 in range(4):
                i = gq * 4 + i4
                pst_ = ps_tail.tile([128, D], DT, tag="tacc")
                nc.tensor.matmul(pst_[:, 0:512], ypb[:, i * 128:(i + 1) * 128],
                                 tailT[:, 0:512], start=True, stop=True)
                nc.tensor.matmul(pst_[:, 512:D], ypb[:, i * 128:(i + 1) * 128],
                                 tailT[:, 512:D], start=True, stop=True)
                drain(osb[:, i4 * D:(i4 + 1) * D], pst_[:])
            nc.sync.dma_start(out=out_r[gq], in_=osb.rearrange("p (i d) -> p i d", d=D))

    if compile:
        nc.compile()
    return nc


def host_prep(inp):
    p = {k: np.ascontiguousarray(np.asarray(v, np.float32)) for k, v in inp.items()}
    s = p["bn_w"] / np.sqrt(p["bn_var"] + 1e-5)
    W1 = (p["head_w"] * s[:, None]).astype(np.float64)
    b1 = (p["head_b"] - p["bn_mean"]) * s + p["bn_b"]
    w = np.arange(W)
    f = np.arange(F)
    h = np.arange(H)
    Cw = np.cos(2 * np.pi * np.outer(w, f) / W) / 8.0
    Sw = -np.sin(2 * np.pi * np.outer(w, f) / W) / 8.0
    Fw2 = np.concatenate([Cw, np.zeros((W, 31)), Sw], axis=1)      # [64, 97]
    Ch = np.cos(2 * np.pi * np.outer(h, h) / H) / 8.0
    Sh = np.sin(2 * np.pi * np.outer(h, h) / H) / 8.0
    cf = np.full(F, 2.0)
    cf[0] = 1.0
    cf[F - 1] = 1.0
    Fwi = np.concatenate([
        (cf[:, None] * np.cos(2 * np.pi * np.outer(f, w) / W)) / 8.0,
        -(cf[:, None] * np.sin(2 * np.pi * np.outer(f, w) / W)) / 8.0,
    ], axis=0)                                                     # [66, 64]
    w9 = (p["conv_A"] @ p["conv_B"])                               # [64, 9]

    cf32 = np.zeros((128, F32_W), np.float32)

    def put32(name, rows, arr):
        arr = np.asarray(arr, np.float32)
        if arr.ndim == 1:
            arr = arr[:, None]
        cf32[rows[0]:rows[0] + arr.shape[0],
             F32_COLS[name]:F32_COLS[name] + arr.shape[1]] = arr

    put32("b1", (0,), b1)
    put32("pb1", (0,), p["proj_b1"])
    put32("cb1", (0,), p["ci_b1"])
    put32("cb2", (64,), p["ci_b2"])
    put32("pb2", (64,), p["proj_b2"])
    put32("cbvec", (64,), p["conv_bias"])
    put32("pw1T", (0,), np.ascontiguousarray(p["proj_w1"].T))
    put32("pw2T", (0,), np.ascontiguousarray(p["proj_w2"].T))
    put32("cw1T", (64,), np.ascontiguousarray(p["ci_w1"].T))
    put32("cw2T", (0,), np.ascontiguousarray(p["ci_w2"].T))
    put32("pb2row", (0,), p["proj_b2"][None, :])

    cbf = np.zeros((128, BF_W), np.float64)

    def putbf(name, rows, arr):
        arr = np.asarray(arr, np.float64)
        if arr.ndim == 1:
            arr = arr[:, None]
        cbf[rows[0]:rows[0] + arr.shape[0],
            BF_COLS[name]:BF_COLS[name] + arr.shape[1]] = arr

    putbf("w1t", (0,), np.ascontiguousarray(W1.T).reshape(NK, 128, HID)
          .transpose(1, 0, 2).reshape(128, NK * HID))
    putbf("fw2", (0,), Fw2)
    putbf("chm", (0,), Ch)
    putbf("chm", (64,), Ch)
    putbf("shm", (0,), Sh)
    putbf("shm", (64,), Sh)
    putbf("nshm", (0,), -Sh)
    putbf("nshm", (64,), -Sh)
    putbf("fwi", (0,), Fwi)
    putbf("ident", (0,), np.eye(128))
    w9d = np.zeros((64, 9 * 64))
    for t_ in range(9):
        w9d[:, t_ * 64:(t_ + 1) * 64] = np.diag(w9[:, t_])
    putbf("w9d", (0,), w9d)
    putbf("tailT", (64,), np.ascontiguousarray(p["tail_w"].T))
    putbf("onesrow", (0,), np.ones((1, 64)))
    putbf("cwv", (0,), np.ascontiguousarray(p["compress_w"].T))
    putbf("siw", (64,), np.ascontiguousarray(p["si_w"].T))

    def spack(name, vec):
        buf = np.zeros((128, NSP))
        for g, (c0, nA, nB) in enumerate(GROUPS):
            base = GBASE[g]
            buf[0:64, base:base + nA * F] = np.repeat(vec[c0:c0 + nA], F)[None, :]
            if nB > 0:
                buf[64:128, base:base + nB * F] = \
                    np.repeat(vec[c0 + nA:c0 + nA + nB], F)[None, :]
        putbf(name, (0,), buf)

    spack("awp", p["amp_w"])
    spack("abp", p["amp_b"])
    spack("pw2p", 2.0 * p["pha_w"])
    spack("pb3p", p["pha_b"])

    consts = dict(
        cf32=np.ascontiguousarray(cf32),
        cbf=np.ascontiguousarray(cbf.astype(ml_dtypes.bfloat16)),
    )
    scalars = dict(
        compress_b=float(p["compress_b"][0]),
        si_b_eff=float(p["si_b"][0] + p["si_w"][0] @ p["conv_bias"]),
    )
    return consts, scalars


def kernel(**inputs):
    x = np.asarray(inputs["x"], np.float32)
    N = x.shape[0]
    consts, scalars = host_prep(inputs)
    nc = build_graph(scalars)
    in_maps = []
    for i in range(N):
        xu = x[i].reshape(HW, D).view(np.uint32)
        xr = ((xu + 0x7FFF + ((xu >> 16) & 1)) >> 16).astype(np.uint16)  # bf16 RNE
        m = {"x": xr.view(ml_dtypes.bfloat16)}
        m.update(consts)
        in_maps.append(m)
    res = run_bass_kernel_spmd(nc, in_maps, core_ids=list(range(N)))
    outs = np.stack([np.asarray(res.results[i]["out"], np.float32).reshape(H, W, D)
                     for i in range(N)])
    return outs.astype(np.float32)
